# revision 2
# baseline (speedup 1.0000x reference)
"""Causal self-attention kernel for Trainium2, 8 NeuronCores — v2.

Problem: B=2, S=2048, D=1024, H=16 heads, Hd=64. fp32 in/out.
Sharding: core c -> batch b=c//4, head-group g=c%4 (4 heads, DL=256 dims).
Row-parallel output projection; host sums 4 partials per batch and adds the
(bv @ Wp.T + bp) correction row (softmax rows sum to 1; bk drops: softmax is
shift-invariant per query).

Speedups over the fp32r baseline (cost-model-guided; numerics-validated):
  - Scores matmul in fp8 DoubleRow: q/k stored x32-scaled fp8 in a
    [128, 2, S] layout whose planes are the low/high 32-dim halves of each
    head (host permutes W-q/k columns so the QKV psum lands directly in
    this layout) -> K=32x2 DoubleRow at 0.5 cyc/col, 2x the bf16 rate.
    fp8 q/k storage measured at 7.6e-3 rel err (gate 2e-2).
  - att@V flipped: P.T is the stationary operand, V the moving one, with a
    single ones-column emitting softmax denominators. The S.T-layout OT
    used only 64 of 128 stationary cols; the flip runs at 65 moving cols
    per 128x128 block = ~2x fewer PE cycles. y lands token-major, so
    normalization is a per-partition-scalar mul (cheap), and a DMA XBAR
    transpose returns y to dim-major for the projection.
  - exp batched per key-chunk PAIR: one ACT instruction covers [128, 2, n]
    across both psum banks of the pair's score tile, halving ACT's
    per-instruction overhead (ACT is the bottleneck engine).
  - causal masks applied post-exp as 0/1 bf16 multiplies on the Pool
    engine (otherwise idle), off ACT's critical path and off DVE.
  - x / v / P / y / Wp in bf16 (fp8 fails the error budget there);
    partial outputs leave as fp16 (halves output DMA; host sums in fp32).
"""
import json
import sys

sys.path.insert(0, "/opt/trn_rl_repo")

import numpy as np
import ml_dtypes

import concourse.bass as bass
import concourse.mybir as mybir
import concourse.tile as tile
from concourse.bass_utils import run_bass_kernel_spmd

F32 = mybir.dt.float32
F16 = mybir.dt.float16
BF16 = mybir.dt.bfloat16
FP8 = mybir.dt.float8e4
NP_FP8 = ml_dtypes.float8_e4m3
NP_BF16 = ml_dtypes.bfloat16
AF = mybir.ActivationFunctionType
OP = mybir.AluOpType
DR = mybir.MatmulPerfMode.DoubleRow

S = 2048          # tokens per batch (= per core)
D = 1024          # model dim
HL = 4            # heads per core
HD = 64           # head dim
DL = HL * HD      # local dims per core (256)
WS = 32.0         # fp8 q/k scale (scores x1024 -> exp scale 2^-13)
EXP_SCALE = 0.125 / (WS * WS)


def _legalize_waits_json(bir_bytes: bytes) -> bytes:
    """walrus allows <=1 sem-wait per instruction (<=2 for EventSemaphore);
    spill extras onto EventSemaphore instructions."""
    j = json.loads(bir_bytes)
    for fn in j["functions"]:
        for bb in fn["blocks"]:
            out = []
            for inst in bb["instructions"]:
                si = inst.get("sync_info") or {}
                ws = si.get("on_wait") or []
                cap = 2 if inst.get("opcode") == "EventSemaphore" else 1
                if len(ws) > cap:
                    extras, keep = ws[:-cap], ws[-cap:]
                    k = 0
                    while extras:
                        chunk, extras = extras[:2], extras[2:]
                        out.append({
                            "debug": inst.get("debug", 0),
                            "engine": inst["engine"],
                            "ins": [],
                            "name": f"{inst['name']}_wfix{k}",
                            "opcode": "EventSemaphore",
                            "outs": [],
                            "sync_info": {"on_update": [], "on_wait": chunk},
                        })
                        k += 1
                    si["on_wait"] = keep
                out.append(inst)
            bb["instructions"] = out
    return json.dumps(j).encode()


def _install_legalizer(nc):
    orig = nc.to_json_bytes
    nc.to_json_bytes = lambda: _legalize_waits_json(orig())


def build_nc() -> bass.Bass:
    nc = bass.Bass(trn_type="TRN2", num_devices=8)

    xT = nc.dram_tensor("xT", [D, S], BF16, kind="ExternalInput")     # x[b].T
    wq = nc.dram_tensor("wq", [D, DL], BF16, kind="ExternalInput")    # perm'd
    wk = nc.dram_tensor("wk", [D, DL], BF16, kind="ExternalInput")    # perm'd
    wv = nc.dram_tensor("wv", [D, DL], BF16, kind="ExternalInput")
    wp = nc.dram_tensor("wp", [DL, D], BF16, kind="ExternalInput")    # Wp.T/32
    bq = nc.dram_tensor("bq", [DL], F32, kind="ExternalInput")        # 32bq
    tri = nc.dram_tensor("tri", [128, 128], BF16, kind="ExternalInput")
    triw = nc.dram_tensor("triw", [128, 256], BF16, kind="ExternalInput")
    ident = nc.dram_tensor("ident", [128, 128], BF16, kind="ExternalInput")
    out = nc.dram_tensor("out", [S, D], F16, kind="ExternalOutput")

    with tile.TileContext(nc) as tc:
        with tc.tile_pool(name="const", bufs=1) as const, \
             tc.tile_pool(name="acts", bufs=1) as acts, \
             tc.tile_pool(name="xin", bufs=2) as xpool, \
             tc.tile_pool(name="pt", bufs=4) as ptp, \
             tc.tile_pool(name="rc", bufs=1) as rcp, \
             tc.tile_pool(name="outp", bufs=3) as outp, \
             tc.tile_pool(name="ps", bufs=1, space="PSUM") as ps:
            wq_sb = const.tile([128, 8, DL], BF16)
            wk_sb = const.tile([128, 8, DL], BF16)
            wv_sb = const.tile([128, 8, DL], BF16)
            wp_sb = const.tile([128, 2, D], BF16)
            bq_sb = const.tile([128, 2], F32)
            tri_sb = const.tile([128, 128], BF16)     # 0/1 causal triangle
            triw_sb = const.tile([128, 256], BF16)    # [zeros | triangle]
            id_sb = const.tile([128, 128], BF16)      # PE-transpose identity

            # q/k: [128, 2, S] fp8; partition 32h+p, plane c = head h's
            # dim 32c+p (weight columns host-permuted to produce this)
            qT_sb = acts.tile([128, 2, S], FP8, name="qT")
            kT_sb = acts.tile([128, 2, S], FP8, name="kT")
            # y token-major: [128 tok, 16 chunks, 256 dims] bf16
            yQ_sb = acts.tile([128, 16, DL], BF16, name="yQ")
            # y dim-major (post DMA-transpose) for the projection
            yT_sb = [acts.tile([128, S], BF16, name=f"yT{m}") for m in range(2)]
            # v per key chunk: [128 keys, 4 heads, 64 v + 1 one]
            vO_sb = [acts.tile([128, HL, HD + 1], BF16, name=f"vO{i}")
                     for i in range(16)]

            x3 = xT[:].rearrange("(kc p) t -> p kc t", p=128)
            wq3 = wq[:].rearrange("(kc p) m -> p kc m", p=128)
            wk3 = wk[:].rearrange("(kc p) m -> p kc m", p=128)
            wv3 = wv[:].rearrange("(kc p) m -> p kc m", p=128)

            xsb_tiles = {}

            def dma_x(t4):
                xsb = xpool.tile([128, 8, 512], BF16, tag="x", name=f"x{t4}")
                xsb_tiles[t4] = xsb
                ts = slice(512 * t4, 512 * t4 + 512)
                for half in range(2):
                    hs = slice(4 * half, 4 * half + 4)
                    nc.sync.dma_start(out=xsb[:, hs, :], in_=x3[:, hs, ts])
                return xsb

            # startup: few LARGE DMAs (the HWDGE queue costs ~600ns per DMA
            # instruction, so many small transfers serialize the prologue).
            # Critical set for the first scores: x0 + full wq/wk (every head
            # reads both dim-half planes); wv next (first att@V), then x1.
            xsb0 = xpool.tile([128, 8, 512], BF16, tag="x", name="x0")
            xsb_tiles[0] = xsb0
            # big input streams on the SP hwdge queue; tiny constants go out
            # on the Activation hwdge queue in parallel (each dma_start costs
            # ~650ns of issue time on its sequencer)
            nc.sync.dma_start(out=id_sb, in_=ident[:])  # first: PE warmup
            nc.scalar.dma_start(out=bq_sb,
                                in_=bq[:].rearrange("(m p) -> p m", p=128))
            nc.scalar.dma_start(out=tri_sb, in_=tri[:])
            nc.scalar.dma_start(out=triw_sb, in_=triw[:])
            nc.sync.dma_start(out=xsb0[:, 0:4, :], in_=x3[:, 0:4, 0:512])
            nc.sync.dma_start(out=wq_sb, in_=wq3)
            nc.sync.dma_start(out=xsb0[:, 4:8, :], in_=x3[:, 4:8, 0:512])
            nc.sync.dma_start(out=wk_sb, in_=wk3)
            nc.sync.dma_start(out=wv_sb, in_=wv3)
            dma_x(1)
            nc.sync.dma_start(
                out=wp_sb, in_=wp[:].rearrange("(m p) n -> p m n", p=128))
            for i in range(16):
                nc.gpsimd.memset(vO_sb[i][:, :, HD:HD + 1], 1.0)

            # p-state warmup: the PE ramps 0.65 -> 1.2 -> 2.4 GHz over 3us of
            # CONTINUOUS execution. Keep it spinning on the identity tile
            # while the x0/wq/wk streams land so the real QKV runs at full
            # clock. (~70 x 128-col transposes span the DMA window.)
            # (pe_busy_start pins at the FIRST PE activity and never resets,
            # so a few early matmuls suffice to have everything 3us+ later
            # run at 2.4GHz)
            warm = ps.tile([128, 4, 128], F32, tag="yps", bufs=2, name="warm")
            wb = warm.bitcast(BF16)
            for i in range(26):
                nc.tensor.transpose(wb[:, 0, 0:128], id_sb, id_sb)

            # --- filler units as GENERATORS: yield between small batches of
            # PE work so the scheduler can trickle them between score pairs
            # without stalling the ST->exp pipeline (ACT is the bottleneck
            # engine; it must never wait on a long filler burst) ---
            def unit_qk(which, t4, m):
                wsb = wq_sb if which == "q" else wk_sb
                dst = qT_sb if which == "q" else kT_sb

                def go():
                    ts512 = slice(512 * t4, 512 * t4 + 512)
                    xsb = xsb_tiles[t4]
                    dsl = slice(128 * m, 128 * m + 128)
                    p = ps.tile([128, 512], F32, tag="mm", bufs=2,
                                name=f"{which}{t4}{m}")
                    for kc in range(8):
                        nc.tensor.matmul(p[:, :], wsb[:, kc, dsl], xsb[:, kc, :],
                                         start=(kc == 0), stop=(kc == 7))
                        if kc % 2 == 1 and kc < 7:
                            yield
                    if which == "q":
                        nc.vector.tensor_scalar_add(dst[:, m, ts512], p,
                                                    bq_sb[:, m:m + 1])
                    else:
                        nc.vector.tensor_copy(dst[:, m, ts512], p)
                return go()

            def unit_v(t4, si):
                def go():
                    xsb = xsb_tiles[t4]
                    tl = slice(128 * si, 128 * si + 128)
                    p = ps.tile([128, DL], F32, tag="mm", bufs=2,
                                name=f"v{t4}{si}")
                    for kc in range(8):
                        nc.tensor.matmul(p[:, :], xsb[:, kc, tl], wv_sb[:, kc, :],
                                         start=(kc == 0), stop=(kc == 7))
                        if kc == 3:
                            yield
                    v3 = vO_sb[4 * t4 + si]
                    nc.vector.tensor_copy(
                        v3[:, :, 0:HD], p[:].rearrange("p (h c) -> p h c", h=HL))
                return go()

            def unit_pj(j, sp, on_act=False):
                """Project 256 tokens; fp16 partials out. on_act: route the
                psum->sbuf copies to ACT (for the tail, when exps are done)."""
                def go():
                    t0 = 512 * j + 256 * sp
                    osb = outp.tile([128, 2, D], F16, tag="o", name=f"o{j}{sp}")
                    for sub in range(2):
                        tsl = slice(t0 + 128 * sub, t0 + 128 * sub + 128)
                        for ncol in range(2):
                            pj = ps.tile([128, 512], F32, tag="mm", bufs=2,
                                         name=f"pj{j}{sp}{sub}{ncol}")
                            for m2 in range(2):
                                nc.tensor.matmul(
                                    pj[:, :], yT_sb[m2][:, tsl],
                                    wp_sb[:, m2, 512 * ncol:512 * ncol + 512],
                                    start=(m2 == 0), stop=(m2 == 1))
                            dst = osb[:, sub, 512 * ncol:512 * ncol + 512]
                            if on_act:
                                nc.scalar.activation(dst, pj, AF.Copy)
                            else:
                                nc.vector.tensor_copy(dst, pj)
                            yield
                        nc.scalar.dma_start(out=out[tsl, :],
                                            in_=osb[:, sub, :])
                return go()

            def unit_tr(j, on_act=False):
                """Transpose yQ chunks of block j into yT via PE is_transpose
                (borrows an mm psum slot bitcast to bf16; the XBAR DMA route
                clogs the HWDGE queue). on_act: tail variant — psum->sbuf
                copies go to ACT, which is idle once the exps are done."""
                def go():
                    for qc in range(4 * j, 4 * j + 4):
                        tp = ps.tile([128, 512], F32, tag="mm", bufs=2,
                                     name=f"tp{qc}")
                        tpb = tp.bitcast(BF16)
                        for m in range(2):
                            nc.tensor.transpose(
                                tpb[:, 128 * m:128 * m + 128],
                                yQ_sb[:, qc, 128 * m:128 * m + 128], id_sb)
                        for m in range(2):
                            dst = yT_sb[m][:, 128 * qc:128 * qc + 128]
                            src = tpb[:, 128 * m:128 * m + 128]
                            if on_act:
                                nc.scalar.activation(dst, src, AF.Copy)
                            else:
                                nc.vector.tensor_copy(dst, src)
                        yield
                return go()

            def drain(gens):
                for g in gens:
                    for _ in g:
                        pass

            # prologue: block 0's q/k (scores read both planes)
            drain([unit_qk(w, 0, m) for w in ("q", "k") for m in range(2)])

            for j in range(4):
                if 2 <= j + 1 < 4:
                    dma_x(j + 1)
                npairs = 2 * (j + 1)
                # rolling filler queue (FIFO of generators); gens with
                # deadlines are tracked by name and force-completed in time
                vgens = [unit_v(j, si) for si in range(4)]
                queue = []
                if j > 0:
                    queue.append(unit_tr(j - 1))
                queue += vgens
                if j > 0:
                    queue += [unit_pj(j - 1, 0), unit_pj(j - 1, 1)]
                if j + 1 < 4:
                    queue += [unit_qk(w, j + 1, m)
                              for w in ("q", "k") for m in range(2)]
                nsteps = {0: 24, 1: 39, 2: 39, 3: 23}[j]
                rate = -(-nsteps // (HL * npairs))
                done = set()

                def pump(steps):
                    while steps > 0 and queue:
                        g = queue[0]
                        try:
                            next(g)
                            steps -= 1
                        except StopIteration:
                            done.add(g)
                            queue.pop(0)

                def force(gens):
                    for g in gens:
                        if g in done:
                            continue
                        for _ in g:
                            pass
                        done.add(g)
                        if g in queue:
                            queue.remove(g)

                def emit_st(h, p):
                    # pair p covers key chunks (2p, 2p+1)
                    hp = slice(32 * h, 32 * h + 32)
                    diag = p >= 2 * j
                    qs = 512 * j if p <= 2 * j else 512 * j + 256
                    n = 512 * j + 512 - qs
                    st = ps.tile([128, 2, 512], F32, tag="st", bufs=2,
                                 name=f"st{j}{h}{p}")
                    for c in range(2):
                        ki = 128 * (2 * p + c)
                        nc.tensor.matmul(st[:, c, 0:n],
                                         kT_sb[hp, :, ki:ki + 128],
                                         qT_sb[hp, :, qs:qs + n],
                                         start=True, stop=True,
                                         perf_mode=DR,
                                         tile_position=(32 * h, 0))
                    pt = ptp.tile([128, 2, 512], BF16, tag="pt",
                                  name=f"pt{j}{h}{p}")
                    nc.scalar.activation(pt[:, :, 0:n], st[:, :, 0:n],
                                         AF.Exp, scale=EXP_SCALE)
                    if diag:
                        # 0/1 masks post-exp, on Pool (keeps ACT/DVE free)
                        nc.gpsimd.tensor_tensor(
                            pt[:, 0, 0:128], pt[:, 0, 0:128], tri_sb,
                            op=OP.mult)
                        nc.gpsimd.tensor_tensor(
                            pt[:, 1, 0:256], pt[:, 1, 0:256], triw_sb,
                            op=OP.mult)
                    return pt, qs, n

                def emit_ot(h, p, yps, pt, qs, n):
                    # flipped att@V: pt stationary, v+ones moving. The four
                    # query-sub accumulation groups share one psum bank;
                    # start=True zeroes the WHOLE 2KB bank (ZERO_REGION_SIZE),
                    # so only the bank's first matmul sets it — the other
                    # groups' first writes land on pending-zero bytes and
                    # overwrite, then accumulate.
                    for qc in range(4):
                        for c in range(2):
                            i = 2 * p + c
                            if i > 4 * j + qc:
                                continue
                            off = 128 * qc + 512 * j - qs
                            if off < 0:
                                continue
                            nc.tensor.matmul(
                                yps[:, qc, 0:HD + 1],
                                pt[:, c, off:off + 128],
                                vO_sb[i][:, h, :],
                                start=(i == 0 and qc == 0),
                                stop=(i == 4 * j + qc),
                                skip_group_check=True)

                def emit_norm(h, yps):
                    # col 64 of each group is the denominator; token-major
                    # layout makes this a per-partition scalar multiply
                    rb = rcp.tile([128, 4], F32, tag=f"rb{h}", name=f"rb{j}{h}")
                    nc.vector.reciprocal(
                        rb, yps[:, :, HD:HD + 1].rearrange("p a b -> p (a b)"))
                    for qc in range(4):
                        nc.vector.tensor_scalar_mul(
                            yQ_sb[:, 4 * j + qc, 64 * h:64 * h + 64],
                            yps[:, qc, 0:HD], rb[:, qc:qc + 1])

                # one flat software-pipelined stream of (h, pair) across all
                # heads: no ACT bubble at h boundaries
                slots = [(h, p) for h in range(HL) for p in range(npairs)]
                LOOK = 2
                pts = {}
                yps_h = {}
                for idx in range(LOOK):
                    pts[idx] = emit_st(*slots[idx])
                for idx, (h, p) in enumerate(slots):
                    if idx + LOOK < len(slots):
                        pts[idx + LOOK] = emit_st(*slots[idx + LOOK])
                    if h == 0:
                        if p == 2 * j:
                            force(vgens[:2])
                        elif p == 2 * j + 1:
                            force(vgens[2:])
                    if p == 0:
                        yps_h[h] = ps.tile([128, 4, 128], F32, tag="yps",
                                           bufs=2, name=f"yps{j}{h}")
                    emit_ot(h, p, yps_h[h], *pts.pop(idx))
                    if p == npairs - 1:
                        emit_norm(h, yps_h.pop(h))
                    if j > 0 or h > 0:  # j0-h0: x1 hasn't landed yet
                        pump(rate)
                pump(10 ** 9)  # flush fillers before the next j block
            drain([unit_tr(3), unit_pj(3, 0), unit_pj(3, 1)])

    _install_legalizer(nc)
    return nc


_NC_CACHE = None


def _get_nc():
    global _NC_CACHE
    if _NC_CACHE is None:
        _NC_CACHE = build_nc()
    return _NC_CACHE


# low/high 32-dim halves of each head -> planes (col 128c+32h+p of the
# permuted weight = dim 64h+32c+p of head h)
_PERM = np.array([64 * h + 32 * c + p
                  for c in range(2) for h in range(4) for p in range(32)])


def make_in_maps(x, Wq, bq, Wk, Wv, Wp):
    x = np.asarray(x, np.float32)
    xT8 = [np.ascontiguousarray(x[b].T).astype(NP_BF16) for b in range(2)]
    t01 = (np.arange(128)[None, :] >= np.arange(128)[:, None])
    tri01 = t01.astype(NP_BF16)
    triw01 = np.concatenate(
        [np.zeros((128, 128), NP_BF16), tri01], axis=1)
    Wq, Wk, Wv, Wp = (np.asarray(w, np.float32) for w in (Wq, Wk, Wv, Wp))
    bq = np.asarray(bq, np.float32)
    in_maps = []
    for c in range(8):
        b, g = c // 4, c % 4
        sl = slice(DL * g, DL * g + DL)
        in_maps.append({
            "xT": xT8[b],
            "wq": np.ascontiguousarray(
                (Wq[sl, :][_PERM, :] * WS).T).astype(NP_BF16),
            "wk": np.ascontiguousarray(
                (Wk[sl, :][_PERM, :] * WS).T).astype(NP_BF16),
            "wv": np.ascontiguousarray((Wv[sl, :] * WS).T).astype(NP_BF16),
            "wp": np.ascontiguousarray(Wp[:, sl].T / WS).astype(NP_BF16),
            "bq": np.ascontiguousarray(bq[sl][_PERM]) * np.float32(WS),
            "tri": tri01,
            "triw": triw01,
            "ident": np.eye(128, dtype=np.float32).astype(NP_BF16),
        })
    return in_maps


def kernel(x, Wq, bq, Wk, bk, Wv, bv, Wp, bp, _run_kwargs=None):
    nc = _get_nc()
    in_maps = make_in_maps(x, Wq, bq, Wk, Wv, Wp)
    res = run_bass_kernel_spmd(nc, in_maps, list(range(8)), **(_run_kwargs or {}))
    corr = (np.asarray(bv, np.float32) @ np.asarray(Wp, np.float32).T
            + np.asarray(bp, np.float32))
    out = np.zeros((2, S, D), np.float32)
    for c in range(8):
        out[c // 4] += np.asarray(res.results[c]["out"], np.float32)
    out += corr[None, None, :]
    kernel.last_results = res
    return out


# revision 3
# speedup vs baseline: 1.0064x; 1.0064x over previous
"""Causal self-attention kernel for Trainium2, 8 NeuronCores — v2.

Problem: B=2, S=2048, D=1024, H=16 heads, Hd=64. fp32 in/out.
Sharding: core c -> batch b=c//4, head-group g=c%4 (4 heads, DL=256 dims).
Row-parallel output projection; host sums 4 partials per batch and adds the
(bv @ Wp.T + bp) correction row (softmax rows sum to 1; bk drops: softmax is
shift-invariant per query).

Speedups over the fp32r baseline (cost-model-guided; numerics-validated):
  - Scores matmul in fp8 DoubleRow: q/k stored x32-scaled fp8 in a
    [128, 2, S] layout whose planes are the low/high 32-dim halves of each
    head (host permutes W-q/k columns so the QKV psum lands directly in
    this layout) -> K=32x2 DoubleRow at 0.5 cyc/col, 2x the bf16 rate.
    fp8 q/k storage measured at 7.6e-3 rel err (gate 2e-2).
  - att@V flipped: P.T is the stationary operand, V the moving one, with a
    single ones-column emitting softmax denominators. The S.T-layout OT
    used only 64 of 128 stationary cols; the flip runs at 65 moving cols
    per 128x128 block = ~2x fewer PE cycles. y lands token-major, so
    normalization is a per-partition-scalar mul (cheap), and a DMA XBAR
    transpose returns y to dim-major for the projection.
  - exp batched per key-chunk PAIR: one ACT instruction covers [128, 2, n]
    across both psum banks of the pair's score tile, halving ACT's
    per-instruction overhead (ACT is the bottleneck engine).
  - causal masks applied post-exp as 0/1 bf16 multiplies on the Pool
    engine (otherwise idle), off ACT's critical path and off DVE.
  - x / v / P / y / Wp in bf16 (fp8 fails the error budget there);
    partial outputs leave as fp16 (halves output DMA; host sums in fp32).
"""
import json
import sys

sys.path.insert(0, "/opt/trn_rl_repo")

import numpy as np
import ml_dtypes

import concourse.bass as bass
import concourse.mybir as mybir
import concourse.tile as tile
from concourse.bass_utils import run_bass_kernel_spmd

F32 = mybir.dt.float32
F16 = mybir.dt.float16
BF16 = mybir.dt.bfloat16
FP8 = mybir.dt.float8e4
NP_FP8 = ml_dtypes.float8_e4m3
NP_BF16 = ml_dtypes.bfloat16
AF = mybir.ActivationFunctionType
OP = mybir.AluOpType
DR = mybir.MatmulPerfMode.DoubleRow

S = 2048          # tokens per batch (= per core)
D = 1024          # model dim
HL = 4            # heads per core
HD = 64           # head dim
DL = HL * HD      # local dims per core (256)
WS = 32.0         # fp8 q/k scale (scores x1024 -> exp scale 2^-13)
EXP_SCALE = 0.125 / (WS * WS)


def _legalize_waits_json(bir_bytes: bytes) -> bytes:
    """walrus allows <=1 sem-wait per instruction (<=2 for EventSemaphore);
    spill extras onto EventSemaphore instructions."""
    j = json.loads(bir_bytes)
    for fn in j["functions"]:
        for bb in fn["blocks"]:
            out = []
            for inst in bb["instructions"]:
                si = inst.get("sync_info") or {}
                ws = si.get("on_wait") or []
                cap = 2 if inst.get("opcode") == "EventSemaphore" else 1
                if len(ws) > cap:
                    extras, keep = ws[:-cap], ws[-cap:]
                    k = 0
                    while extras:
                        chunk, extras = extras[:2], extras[2:]
                        out.append({
                            "debug": inst.get("debug", 0),
                            "engine": inst["engine"],
                            "ins": [],
                            "name": f"{inst['name']}_wfix{k}",
                            "opcode": "EventSemaphore",
                            "outs": [],
                            "sync_info": {"on_update": [], "on_wait": chunk},
                        })
                        k += 1
                    si["on_wait"] = keep
                out.append(inst)
            bb["instructions"] = out
    return json.dumps(j).encode()


def _install_legalizer(nc):
    orig = nc.to_json_bytes
    nc.to_json_bytes = lambda: _legalize_waits_json(orig())


def build_nc() -> bass.Bass:
    nc = bass.Bass(trn_type="TRN2", num_devices=8)

    xT = nc.dram_tensor("xT", [D, S], BF16, kind="ExternalInput")     # x[b].T
    wq = nc.dram_tensor("wq", [D, DL], BF16, kind="ExternalInput")    # perm'd
    wk = nc.dram_tensor("wk", [D, DL], BF16, kind="ExternalInput")    # perm'd
    wv = nc.dram_tensor("wv", [D, DL], BF16, kind="ExternalInput")
    wp = nc.dram_tensor("wp", [DL, D], BF16, kind="ExternalInput")    # Wp.T/32
    bq = nc.dram_tensor("bq", [DL], F32, kind="ExternalInput")        # 32bq
    tri = nc.dram_tensor("tri", [128, 128], BF16, kind="ExternalInput")
    triw = nc.dram_tensor("triw", [128, 256], BF16, kind="ExternalInput")
    ident = nc.dram_tensor("ident", [128, 128], BF16, kind="ExternalInput")
    out = nc.dram_tensor("out", [S, D], F16, kind="ExternalOutput")

    with tile.TileContext(nc) as tc:
        with tc.tile_pool(name="const", bufs=1) as const, \
             tc.tile_pool(name="acts", bufs=1) as acts, \
             tc.tile_pool(name="xin", bufs=2) as xpool, \
             tc.tile_pool(name="pt", bufs=4) as ptp, \
             tc.tile_pool(name="rc", bufs=1) as rcp, \
             tc.tile_pool(name="outp", bufs=3) as outp, \
             tc.tile_pool(name="ps", bufs=1, space="PSUM") as ps:
            wq_sb = const.tile([128, 8, DL], BF16)
            wk_sb = const.tile([128, 8, DL], BF16)
            wv_sb = const.tile([128, 8, DL], BF16)
            wp_sb = const.tile([128, 2, D], BF16)
            bq_sb = const.tile([128, 2], F32)
            tri_sb = const.tile([128, 128], BF16)     # 0/1 causal triangle
            triw_sb = const.tile([128, 256], BF16)    # [zeros | triangle]
            id_sb = const.tile([128, 128], BF16)      # PE-transpose identity

            # q/k: [128, 2, S] fp8; partition 32h+p, plane c = head h's
            # dim 32c+p (weight columns host-permuted to produce this)
            qT_sb = acts.tile([128, 2, S], FP8, name="qT")
            kT_sb = acts.tile([128, 2, S], FP8, name="kT")
            # y token-major: [128 tok, 16 chunks, 256 dims] bf16
            yQ_sb = acts.tile([128, 16, DL], BF16, name="yQ")
            # y dim-major (post DMA-transpose) for the projection
            yT_sb = [acts.tile([128, S], BF16, name=f"yT{m}") for m in range(2)]
            # v per key chunk: [128 keys, 4 heads, 64 v + 1 one]
            vO_sb = [acts.tile([128, HL, HD + 1], BF16, name=f"vO{i}")
                     for i in range(16)]

            x3 = xT[:].rearrange("(kc p) t -> p kc t", p=128)
            wq3 = wq[:].rearrange("(kc p) m -> p kc m", p=128)
            wk3 = wk[:].rearrange("(kc p) m -> p kc m", p=128)
            wv3 = wv[:].rearrange("(kc p) m -> p kc m", p=128)

            xsb_tiles = {}

            def dma_x(t4):
                xsb = xpool.tile([128, 8, 512], BF16, tag="x", name=f"x{t4}")
                xsb_tiles[t4] = xsb
                ts = slice(512 * t4, 512 * t4 + 512)
                nc.sync.dma_start(out=xsb, in_=x3[:, :, ts])
                return xsb

            # startup: few LARGE DMAs (the HWDGE queue costs ~600ns per DMA
            # instruction, so many small transfers serialize the prologue).
            # Critical set for the first scores: x0 + full wq/wk (every head
            # reads both dim-half planes); wv next (first att@V), then x1.
            xsb0 = xpool.tile([128, 8, 512], BF16, tag="x", name="x0")
            xsb_tiles[0] = xsb0
            # big input streams on the SP hwdge queue; tiny constants go out
            # on the Activation hwdge queue in parallel (each dma_start costs
            # ~650ns of issue time on its sequencer)
            nc.sync.dma_start(out=id_sb, in_=ident[:])  # first: PE warmup
            nc.scalar.dma_start(out=bq_sb,
                                in_=bq[:].rearrange("(m p) -> p m", p=128))
            nc.scalar.dma_start(out=tri_sb, in_=tri[:])
            nc.scalar.dma_start(out=triw_sb, in_=triw[:])
            nc.sync.dma_start(out=xsb0[:, 0:4, :], in_=x3[:, 0:4, 0:512])
            nc.sync.dma_start(out=wq_sb, in_=wq3)
            nc.sync.dma_start(out=xsb0[:, 4:8, :], in_=x3[:, 4:8, 0:512])
            nc.sync.dma_start(out=wk_sb, in_=wk3)
            nc.sync.dma_start(out=wv_sb, in_=wv3)
            dma_x(1)
            nc.sync.dma_start(
                out=wp_sb, in_=wp[:].rearrange("(m p) n -> p m n", p=128))
            for i in range(16):
                nc.gpsimd.memset(vO_sb[i][:, :, HD:HD + 1], 1.0)

            # p-state warmup: the PE ramps 0.65 -> 1.2 -> 2.4 GHz over 3us of
            # CONTINUOUS execution. Keep it spinning on the identity tile
            # while the x0/wq/wk streams land so the real QKV runs at full
            # clock. (~70 x 128-col transposes span the DMA window.)
            # preload the ACT Exp table during the DMA wait: the first real
            # exp otherwise pays the 1283ns table load on the critical path
            scr = const.tile([128, 1], F32)
            nc.vector.memset(scr, 0.0)
            nc.scalar.activation(scr, scr, AF.Exp, scale=0.0)

            # (pe_busy_start pins at the FIRST PE activity and never resets,
            # so a few early matmuls suffice to have everything 3us+ later
            # run at 2.4GHz)
            warm = ps.tile([128, 4, 128], F32, tag="yps", bufs=2, name="warm")
            wb = warm.bitcast(BF16)
            for i in range(40):
                nc.tensor.transpose(wb[:, 0, 0:128], id_sb, id_sb)

            # --- filler units as GENERATORS: yield between small batches of
            # PE work so the scheduler can trickle them between score pairs
            # without stalling the ST->exp pipeline (ACT is the bottleneck
            # engine; it must never wait on a long filler burst) ---
            def unit_qk(which, t4, m, t0=0, tn=512):
                wsb = wq_sb if which == "q" else wk_sb
                dst = qT_sb if which == "q" else kT_sb

                def go():
                    tsl = slice(512 * t4 + t0, 512 * t4 + t0 + tn)
                    xsb = xsb_tiles[t4]
                    dsl = slice(128 * m, 128 * m + 128)
                    p = ps.tile([128, 512], F32, tag="mm", bufs=2,
                                name=f"{which}{t4}{m}{t0}")
                    for kc in range(8):
                        nc.tensor.matmul(p[:, 0:tn], wsb[:, kc, dsl],
                                         xsb[:, kc, t0:t0 + tn],
                                         start=(kc == 0), stop=(kc == 7))
                        if kc % 2 == 1 and kc < 7:
                            yield
                    if which == "q":
                        nc.vector.tensor_scalar_add(dst[:, m, tsl], p[:, 0:tn],
                                                    bq_sb[:, m:m + 1])
                    else:
                        nc.vector.tensor_copy(dst[:, m, tsl], p[:, 0:tn])
                return go()

            def unit_v(t4, si):
                def go():
                    xsb = xsb_tiles[t4]
                    tl = slice(128 * si, 128 * si + 128)
                    p = ps.tile([128, DL], F32, tag="mm", bufs=2,
                                name=f"v{t4}{si}")
                    for kc in range(8):
                        nc.tensor.matmul(p[:, :], xsb[:, kc, tl], wv_sb[:, kc, :],
                                         start=(kc == 0), stop=(kc == 7))
                        if kc == 3:
                            yield
                    v3 = vO_sb[4 * t4 + si]
                    nc.vector.tensor_copy(
                        v3[:, :, 0:HD], p[:].rearrange("p (h c) -> p h c", h=HL))
                return go()

            def unit_pj(j, sp, on_act=False):
                """Project 256 tokens; fp16 partials out. on_act: route the
                psum->sbuf copies to ACT (for the tail, when exps are done)."""
                def go():
                    t0 = 512 * j + 256 * sp
                    osb = outp.tile([128, 2, D], F16, tag="o", name=f"o{j}{sp}")
                    for sub in range(2):
                        tsl = slice(t0 + 128 * sub, t0 + 128 * sub + 128)
                        for ncol in range(2):
                            pj = ps.tile([128, 512], F32, tag="mm", bufs=2,
                                         name=f"pj{j}{sp}{sub}{ncol}")
                            for m2 in range(2):
                                nc.tensor.matmul(
                                    pj[:, :], yT_sb[m2][:, tsl],
                                    wp_sb[:, m2, 512 * ncol:512 * ncol + 512],
                                    start=(m2 == 0), stop=(m2 == 1))
                            dst = osb[:, sub, 512 * ncol:512 * ncol + 512]
                            if on_act:
                                nc.scalar.activation(dst, pj, AF.Copy)
                            else:
                                nc.vector.tensor_copy(dst, pj)
                            yield
                        nc.scalar.dma_start(out=out[tsl, :],
                                            in_=osb[:, sub, :])
                return go()

            def unit_tr(j, on_act=False):
                """Transpose yQ chunks of block j into yT via PE is_transpose
                (borrows an mm psum slot bitcast to bf16; the XBAR DMA route
                clogs the HWDGE queue). on_act: tail variant — psum->sbuf
                copies go to ACT, which is idle once the exps are done."""
                def go():
                    for qc in range(4 * j, 4 * j + 4):
                        tp = ps.tile([128, 512], F32, tag="mm", bufs=2,
                                     name=f"tp{qc}")
                        tpb = tp.bitcast(BF16)
                        for m in range(2):
                            nc.tensor.transpose(
                                tpb[:, 128 * m:128 * m + 128],
                                yQ_sb[:, qc, 128 * m:128 * m + 128], id_sb)
                        for m in range(2):
                            dst = yT_sb[m][:, 128 * qc:128 * qc + 128]
                            src = tpb[:, 128 * m:128 * m + 128]
                            if on_act:
                                nc.scalar.activation(dst, src, AF.Copy)
                            else:
                                nc.vector.tensor_copy(dst, src)
                        yield
                return go()

            def drain(gens):
                for g in gens:
                    for _ in g:
                        pass

            # prologue: block 0's q, then only the first 256 key-tokens of k
            # — enough for the first score pair; the second k half follows
            # while that pair's exp runs
            drain([unit_qk("q", 0, 0), unit_qk("q", 0, 1),
                   unit_qk("k", 0, 0, 0, 256), unit_qk("k", 0, 1, 0, 256)])
            k2nd = [unit_qk("k", 0, 0, 256, 256), unit_qk("k", 0, 1, 256, 256)]

            for j in range(4):
                if 2 <= j + 1 < 4:
                    dma_x(j + 1)
                npairs = 2 * (j + 1)
                # rolling filler queue (FIFO of generators); gens with
                # deadlines are tracked by name and force-completed in time
                vgens = [unit_v(j, si) for si in range(4)]
                queue = []
                if j > 0:
                    queue.append(unit_tr(j - 1))
                queue += vgens
                if j > 0:
                    queue += [unit_pj(j - 1, 0), unit_pj(j - 1, 1)]
                if j + 1 < 4:
                    queue += [unit_qk(w, j + 1, m)
                              for w in ("q", "k") for m in range(2)]
                nsteps = {0: 24, 1: 39, 2: 39, 3: 23}[j]
                rate = -(-nsteps // (HL * npairs))
                done = set()

                def pump(steps):
                    while steps > 0 and queue:
                        g = queue[0]
                        try:
                            next(g)
                            steps -= 1
                        except StopIteration:
                            done.add(g)
                            queue.pop(0)

                def force(gens):
                    for g in gens:
                        if g in done:
                            continue
                        for _ in g:
                            pass
                        done.add(g)
                        if g in queue:
                            queue.remove(g)

                def emit_st(h, p):
                    # pair p covers key chunks (2p, 2p+1)
                    hp = slice(32 * h, 32 * h + 32)
                    diag = p >= 2 * j
                    qs = 512 * j if p <= 2 * j else 512 * j + 256
                    n = 512 * j + 512 - qs
                    st = ps.tile([128, 2, 512], F32, tag="st", bufs=2,
                                 name=f"st{j}{h}{p}")
                    for c in range(2):
                        ki = 128 * (2 * p + c)
                        nc.tensor.matmul(st[:, c, 0:n],
                                         kT_sb[hp, :, ki:ki + 128],
                                         qT_sb[hp, :, qs:qs + n],
                                         start=True, stop=True,
                                         perf_mode=DR,
                                         tile_position=(32 * h, 0))
                    pt = ptp.tile([128, 2, 512], BF16, tag="pt",
                                  name=f"pt{j}{h}{p}")
                    nc.scalar.activation(pt[:, :, 0:n], st[:, :, 0:n],
                                         AF.Exp, scale=EXP_SCALE)
                    if diag:
                        # 0/1 masks post-exp, on Pool (keeps ACT/DVE free)
                        nc.gpsimd.tensor_tensor(
                            pt[:, 0, 0:128], pt[:, 0, 0:128], tri_sb,
                            op=OP.mult)
                        nc.gpsimd.tensor_tensor(
                            pt[:, 1, 0:256], pt[:, 1, 0:256], triw_sb,
                            op=OP.mult)
                    return pt, qs, n

                def emit_ot(h, p, yps, pt, qs, n):
                    # flipped att@V: pt stationary, v+ones moving. The four
                    # query-sub accumulation groups share one psum bank;
                    # start=True zeroes the WHOLE 2KB bank (ZERO_REGION_SIZE),
                    # so only the bank's first matmul sets it — the other
                    # groups' first writes land on pending-zero bytes and
                    # overwrite, then accumulate.
                    for qc in range(4):
                        for c in range(2):
                            i = 2 * p + c
                            if i > 4 * j + qc:
                                continue
                            off = 128 * qc + 512 * j - qs
                            if off < 0:
                                continue
                            nc.tensor.matmul(
                                yps[:, qc, 0:HD + 1],
                                pt[:, c, off:off + 128],
                                vO_sb[i][:, h, :],
                                start=(i == 0 and qc == 0),
                                stop=(i == 4 * j + qc),
                                skip_group_check=True)

                def emit_norm(h, yps):
                    # col 64 of each group is the denominator; token-major
                    # layout makes this a per-partition scalar multiply
                    rb = rcp.tile([128, 4], F32, tag=f"rb{h}", name=f"rb{j}{h}")
                    nc.vector.reciprocal(
                        rb, yps[:, :, HD:HD + 1].rearrange("p a b -> p (a b)"))
                    for qc in range(4):
                        nc.vector.tensor_scalar_mul(
                            yQ_sb[:, 4 * j + qc, 64 * h:64 * h + 64],
                            yps[:, qc, 0:HD], rb[:, qc:qc + 1])

                # one flat software-pipelined stream of (h, pair) across all
                # heads: no ACT bubble at h boundaries
                slots = [(h, p) for h in range(HL) for p in range(npairs)]
                LOOK = 2
                pts = {}
                yps_h = {}
                pts[0] = emit_st(*slots[0])
                if j == 0:  # second k half lands while pair 0's exp runs
                    drain(k2nd)
                pts[1] = emit_st(*slots[1])
                for idx, (h, p) in enumerate(slots):
                    if idx + LOOK < len(slots):
                        pts[idx + LOOK] = emit_st(*slots[idx + LOOK])
                    if h == 0:
                        if p == 2 * j:
                            force(vgens[:2])
                        elif p == 2 * j + 1:
                            force(vgens[2:])
                    if p == 0:
                        yps_h[h] = ps.tile([128, 4, 128], F32, tag="yps",
                                           bufs=2, name=f"yps{j}{h}")
                    emit_ot(h, p, yps_h[h], *pts.pop(idx))
                    if p == npairs - 1:
                        emit_norm(h, yps_h.pop(h))
                    if j > 0 or h > 0:  # j0-h0: x1 hasn't landed yet
                        pump(rate)
                pump(10 ** 9)  # flush fillers before the next j block
            drain([unit_tr(3), unit_pj(3, 0), unit_pj(3, 1)])

    _install_legalizer(nc)
    return nc


_NC_CACHE = None


def _get_nc():
    global _NC_CACHE
    if _NC_CACHE is None:
        _NC_CACHE = build_nc()
    return _NC_CACHE


# low/high 32-dim halves of each head -> planes (col 128c+32h+p of the
# permuted weight = dim 64h+32c+p of head h)
_PERM = np.array([64 * h + 32 * c + p
                  for c in range(2) for h in range(4) for p in range(32)])


def make_in_maps(x, Wq, bq, Wk, Wv, Wp):
    x = np.asarray(x, np.float32)
    xT8 = [np.ascontiguousarray(x[b].T).astype(NP_BF16) for b in range(2)]
    t01 = (np.arange(128)[None, :] >= np.arange(128)[:, None])
    tri01 = t01.astype(NP_BF16)
    triw01 = np.concatenate(
        [np.zeros((128, 128), NP_BF16), tri01], axis=1)
    Wq, Wk, Wv, Wp = (np.asarray(w, np.float32) for w in (Wq, Wk, Wv, Wp))
    bq = np.asarray(bq, np.float32)
    in_maps = []
    for c in range(8):
        b, g = c // 4, c % 4
        sl = slice(DL * g, DL * g + DL)
        in_maps.append({
            "xT": xT8[b],
            "wq": np.ascontiguousarray(
                (Wq[sl, :][_PERM, :] * WS).T).astype(NP_BF16),
            "wk": np.ascontiguousarray(
                (Wk[sl, :][_PERM, :] * WS).T).astype(NP_BF16),
            "wv": np.ascontiguousarray((Wv[sl, :] * WS).T).astype(NP_BF16),
            "wp": np.ascontiguousarray(Wp[:, sl].T / WS).astype(NP_BF16),
            "bq": np.ascontiguousarray(bq[sl][_PERM]) * np.float32(WS),
            "tri": tri01,
            "triw": triw01,
            "ident": np.eye(128, dtype=np.float32).astype(NP_BF16),
        })
    return in_maps


def kernel(x, Wq, bq, Wk, bk, Wv, bv, Wp, bp, _run_kwargs=None):
    nc = _get_nc()
    in_maps = make_in_maps(x, Wq, bq, Wk, Wv, Wp)
    res = run_bass_kernel_spmd(nc, in_maps, list(range(8)), **(_run_kwargs or {}))
    corr = (np.asarray(bv, np.float32) @ np.asarray(Wp, np.float32).T
            + np.asarray(bp, np.float32))
    out = np.zeros((2, S, D), np.float32)
    for c in range(8):
        out[c // 4] += np.asarray(res.results[c]["out"], np.float32)
    out += corr[None, None, :]
    kernel.last_results = res
    return out


# revision 4
# speedup vs baseline: 1.0443x; 1.0376x over previous
"""Causal self-attention kernel for Trainium2, 8 NeuronCores — v2.

Problem: B=2, S=2048, D=1024, H=16 heads, Hd=64. fp32 in/out.
Sharding: core c -> batch b=c//4, head-group g=c%4 (4 heads, DL=256 dims).
Row-parallel output projection; host sums 4 partials per batch and adds the
(bv @ Wp.T + bp) correction row (softmax rows sum to 1; bk drops: softmax is
shift-invariant per query).

Speedups over the fp32r baseline (cost-model-guided; numerics-validated):
  - Scores matmul in fp8 DoubleRow: q/k stored x32-scaled fp8 in a
    [128, 2, S] layout whose planes are the low/high 32-dim halves of each
    head (host permutes W-q/k columns so the QKV psum lands directly in
    this layout) -> K=32x2 DoubleRow at 0.5 cyc/col, 2x the bf16 rate.
    fp8 q/k storage measured at 7.6e-3 rel err (gate 2e-2).
  - att@V flipped: P.T is the stationary operand, V the moving one, with a
    single ones-column emitting softmax denominators. The S.T-layout OT
    used only 64 of 128 stationary cols; the flip runs at 65 moving cols
    per 128x128 block = ~2x fewer PE cycles. y lands token-major, so
    normalization is a per-partition-scalar mul (cheap), and a DMA XBAR
    transpose returns y to dim-major for the projection.
  - exp batched per key-chunk PAIR: one ACT instruction covers [128, 2, n]
    across both psum banks of the pair's score tile, halving ACT's
    per-instruction overhead (ACT is the bottleneck engine).
  - causal masks applied post-exp as 0/1 bf16 multiplies on the Pool
    engine (otherwise idle), off ACT's critical path and off DVE.
  - x / v / P / y / Wp in bf16 (fp8 fails the error budget there);
    partial outputs leave as fp16 (halves output DMA; host sums in fp32).
"""
import json
import sys

sys.path.insert(0, "/opt/trn_rl_repo")

import numpy as np
import ml_dtypes

import concourse.bass as bass
import concourse.mybir as mybir
import concourse.tile as tile
from concourse.bass_utils import run_bass_kernel_spmd

F32 = mybir.dt.float32
F16 = mybir.dt.float16
BF16 = mybir.dt.bfloat16
FP8 = mybir.dt.float8e4
NP_FP8 = ml_dtypes.float8_e4m3
NP_BF16 = ml_dtypes.bfloat16
AF = mybir.ActivationFunctionType
OP = mybir.AluOpType
DR = mybir.MatmulPerfMode.DoubleRow

S = 2048          # tokens per batch (= per core)
D = 1024          # model dim
HL = 4            # heads per core
HD = 64           # head dim
DL = HL * HD      # local dims per core (256)
WS = 32.0         # fp8 q/k scale (scores x1024 -> exp scale 2^-13)
EXP_SCALE = 0.125 / (WS * WS)


def _legalize_waits_json(bir_bytes: bytes) -> bytes:
    """walrus allows <=1 sem-wait per instruction (<=2 for EventSemaphore);
    spill extras onto EventSemaphore instructions."""
    j = json.loads(bir_bytes)
    for fn in j["functions"]:
        for bb in fn["blocks"]:
            out = []
            for inst in bb["instructions"]:
                si = inst.get("sync_info") or {}
                ws = si.get("on_wait") or []
                cap = 2 if inst.get("opcode") == "EventSemaphore" else 1
                if len(ws) > cap:
                    extras, keep = ws[:-cap], ws[-cap:]
                    k = 0
                    while extras:
                        chunk, extras = extras[:2], extras[2:]
                        out.append({
                            "debug": inst.get("debug", 0),
                            "engine": inst["engine"],
                            "ins": [],
                            "name": f"{inst['name']}_wfix{k}",
                            "opcode": "EventSemaphore",
                            "outs": [],
                            "sync_info": {"on_update": [], "on_wait": chunk},
                        })
                        k += 1
                    si["on_wait"] = keep
                out.append(inst)
            bb["instructions"] = out
    return json.dumps(j).encode()


def _install_legalizer(nc):
    orig = nc.to_json_bytes
    nc.to_json_bytes = lambda: _legalize_waits_json(orig())


def build_nc() -> bass.Bass:
    nc = bass.Bass(trn_type="TRN2", num_devices=8)

    # compensated fp8 QKV: projections accumulate x8@w8 + xr16@(w8/16) +
    # (x8/16)@wr16 in ONE psum group (the 1/16 scales are folded into
    # pre-scaled fp8 operand copies), 12 DoubleRow matmuls at 2x bf16 rate
    xT = nc.dram_tensor("xT", [D, S], FP8, kind="ExternalInput")      # x8
    xTr = nc.dram_tensor("xTr", [D, S], FP8, kind="ExternalInput")    # 16*res
    xTd = nc.dram_tensor("xTd", [D, S], FP8, kind="ExternalInput")    # x8/16
    wq = nc.dram_tensor("wq", [D, 3, DL], FP8, kind="ExternalInput")  # perm'd
    wk = nc.dram_tensor("wk", [D, 3, DL], FP8, kind="ExternalInput")  # perm'd
    wv = nc.dram_tensor("wv", [D, 3, DL], FP8, kind="ExternalInput")
    wp = nc.dram_tensor("wp", [DL, D], BF16, kind="ExternalInput")    # Wp.T/32
    bq = nc.dram_tensor("bq", [DL], F32, kind="ExternalInput")        # 32bq
    tri = nc.dram_tensor("tri", [128, 128], BF16, kind="ExternalInput")
    triw = nc.dram_tensor("triw", [128, 256], BF16, kind="ExternalInput")
    ident = nc.dram_tensor("ident", [128, 128], BF16, kind="ExternalInput")
    out = nc.dram_tensor("out", [S, D], F16, kind="ExternalOutput")

    with tile.TileContext(nc) as tc:
        with tc.tile_pool(name="const", bufs=1) as const, \
             tc.tile_pool(name="acts", bufs=1) as acts, \
             tc.tile_pool(name="xin", bufs=2) as xpool, \
             tc.tile_pool(name="pt", bufs=4) as ptp, \
             tc.tile_pool(name="rc", bufs=1) as rcp, \
             tc.tile_pool(name="outp", bufs=3) as outp, \
             tc.tile_pool(name="ps", bufs=1, space="PSUM") as ps:
            wq_sb = const.tile([128, 8, 3, DL], FP8)
            wk_sb = const.tile([128, 8, 3, DL], FP8)
            wv_sb = const.tile([128, 8, 3, DL], FP8)
            wp_sb = const.tile([128, 2, D], BF16)
            bq_sb = const.tile([128, 2], F32)
            tri_sb = const.tile([128, 128], BF16)     # 0/1 causal triangle
            triw_sb = const.tile([128, 256], BF16)    # [zeros | triangle]
            id_sb = const.tile([128, 128], BF16)      # PE-transpose identity

            # q/k: [128, 2, S] fp8; partition 32h+p, plane c = head h's
            # dim 32c+p (weight columns host-permuted to produce this)
            qT_sb = acts.tile([128, 2, S], FP8, name="qT")
            kT_sb = acts.tile([128, 2, S], FP8, name="kT")
            # y token-major: [128 tok, 16 chunks, 256 dims] bf16
            yQ_sb = acts.tile([128, 16, DL], BF16, name="yQ")
            # y dim-major (post DMA-transpose) for the projection
            yT_sb = [acts.tile([128, S], BF16, name=f"yT{m}") for m in range(2)]
            # v per key chunk: [128 keys, 4 heads, 64 v + 1 one]
            vO_sb = [acts.tile([128, HL, HD + 1], BF16, name=f"vO{i}")
                     for i in range(16)]

            x3 = xT[:].rearrange("(kc p) t -> p kc t", p=128)
            xr3 = xTr[:].rearrange("(kc p) t -> p kc t", p=128)
            xd3 = xTd[:].rearrange("(kc p) t -> p kc t", p=128)
            wq3 = wq[:].rearrange("(kc p) r m -> p kc r m", p=128)
            wk3 = wk[:].rearrange("(kc p) r m -> p kc r m", p=128)
            wv3 = wv[:].rearrange("(kc p) r m -> p kc r m", p=128)

            xsb_tiles = {}

            def dma_x(t4):
                xsb = xpool.tile([128, 8, 512], FP8, tag="x8", name=f"x{t4}")
                xrb = xpool.tile([128, 8, 512], FP8, tag="xr", name=f"xr{t4}")
                xdb = xpool.tile([128, 8, 512], FP8, tag="xd", name=f"xd{t4}")
                xsb_tiles[t4] = (xsb, xrb, xdb)
                ts = slice(512 * t4, 512 * t4 + 512)
                nc.sync.dma_start(out=xsb, in_=x3[:, :, ts])
                nc.sync.dma_start(out=xrb, in_=xr3[:, :, ts])
                nc.sync.dma_start(out=xdb, in_=xd3[:, :, ts])
                return xsb

            # startup: few LARGE DMAs (the HWDGE queue costs ~600ns per DMA
            # instruction, so many small transfers serialize the prologue).
            # Critical set for the first scores: x0 + full wq/wk (every head
            # reads both dim-half planes); wv next (first att@V), then x1.
            xsb0 = xpool.tile([128, 8, 512], FP8, tag="x8", name="x0")
            xrb0 = xpool.tile([128, 8, 512], FP8, tag="xr", name="xr0")
            xdb0 = xpool.tile([128, 8, 512], FP8, tag="xd", name="xd0")
            xsb_tiles[0] = (xsb0, xrb0, xdb0)
            # big input streams on the SP hwdge queue; tiny constants go out
            # on the Activation hwdge queue in parallel (each dma_start costs
            # ~650ns of issue time on its sequencer)
            nc.sync.dma_start(out=id_sb, in_=ident[:])  # first: PE warmup
            nc.scalar.dma_start(out=bq_sb,
                                in_=bq[:].rearrange("(m p) -> p m", p=128))
            nc.scalar.dma_start(out=tri_sb, in_=tri[:])
            nc.scalar.dma_start(out=triw_sb, in_=triw[:])
            nc.sync.dma_start(out=xsb0, in_=x3[:, :, 0:512])
            nc.sync.dma_start(out=wq_sb, in_=wq3)
            nc.sync.dma_start(out=xrb0, in_=xr3[:, :, 0:512])
            nc.sync.dma_start(out=xdb0, in_=xd3[:, :, 0:512])
            nc.sync.dma_start(out=wk_sb, in_=wk3)
            nc.sync.dma_start(out=wv_sb, in_=wv3)
            dma_x(1)
            nc.sync.dma_start(
                out=wp_sb, in_=wp[:].rearrange("(m p) n -> p m n", p=128))
            for i in range(16):
                nc.gpsimd.memset(vO_sb[i][:, :, HD:HD + 1], 1.0)

            # p-state warmup: the PE ramps 0.65 -> 1.2 -> 2.4 GHz over 3us of
            # CONTINUOUS execution. Keep it spinning on the identity tile
            # while the x0/wq/wk streams land so the real QKV runs at full
            # clock. (~70 x 128-col transposes span the DMA window.)
            # preload the ACT Exp table during the DMA wait: the first real
            # exp otherwise pays the 1283ns table load on the critical path
            scr = const.tile([128, 1], F32)
            nc.vector.memset(scr, 0.0)
            nc.scalar.activation(scr, scr, AF.Exp, scale=0.0)

            # (pe_busy_start pins at the FIRST PE activity and never resets,
            # so a few early matmuls suffice to have everything 3us+ later
            # run at 2.4GHz)
            warm = ps.tile([128, 4, 128], F32, tag="yps", bufs=2, name="warm")
            wb = warm.bitcast(BF16)
            for i in range(40):
                nc.tensor.transpose(wb[:, 0, 0:128], id_sb, id_sb)

            # --- filler units as GENERATORS: yield between small batches of
            # PE work so the scheduler can trickle them between score pairs
            # without stalling the ST->exp pipeline (ACT is the bottleneck
            # engine; it must never wait on a long filler burst) ---
            def unit_qk(which, t4, m, t0=0, tn=512):
                wsb = wq_sb if which == "q" else wk_sb
                dst = qT_sb if which == "q" else kT_sb

                def go():
                    tsl = slice(512 * t4 + t0, 512 * t4 + t0 + tn)
                    x8, xr, xd = xsb_tiles[t4]
                    dsl = slice(128 * m, 128 * m + 128)
                    p = ps.tile([128, 512], F32, tag="mm", bufs=2,
                                name=f"{which}{t4}{m}{t0}")
                    for r, xop in ((0, x8), (1, xr), (2, xd)):
                        for kp in range(4):
                            kk = slice(2 * kp, 2 * kp + 2)
                            nc.tensor.matmul(p[:, 0:tn], wsb[:, kk, r, dsl],
                                             xop[:, kk, t0:t0 + tn],
                                             start=(r == 0 and kp == 0),
                                             stop=(r == 2 and kp == 3),
                                             perf_mode=DR)
                            if kp % 2 == 1 and (r, kp) != (2, 3):
                                yield
                    if which == "q":
                        nc.vector.tensor_scalar_add(dst[:, m, tsl], p[:, 0:tn],
                                                    bq_sb[:, m:m + 1])
                    else:
                        nc.vector.tensor_copy(dst[:, m, tsl], p[:, 0:tn])
                return go()

            def unit_v(t4, si):
                def go():
                    x8, xr, xd = xsb_tiles[t4]
                    tl = slice(128 * si, 128 * si + 128)
                    p = ps.tile([128, DL], F32, tag="mm", bufs=2,
                                name=f"v{t4}{si}")
                    for r, xop in ((0, x8), (1, xr), (2, xd)):
                        for kp in range(4):
                            kk = slice(2 * kp, 2 * kp + 2)
                            nc.tensor.matmul(p[:, :], xop[:, kk, tl],
                                             wv_sb[:, kk, r, :],
                                             start=(r == 0 and kp == 0),
                                             stop=(r == 2 and kp == 3),
                                             perf_mode=DR)
                        if r < 2:
                            yield
                    v3 = vO_sb[4 * t4 + si]
                    nc.vector.tensor_copy(
                        v3[:, :, 0:HD], p[:].rearrange("p (h c) -> p h c", h=HL))
                return go()

            def unit_pj(j, sp, on_act=False):
                """Project 256 tokens; fp16 partials out. on_act: route the
                psum->sbuf copies to ACT (for the tail, when exps are done)."""
                def go():
                    t0 = 512 * j + 256 * sp
                    osb = outp.tile([128, 2, D], F16, tag="o", name=f"o{j}{sp}")
                    for sub in range(2):
                        tsl = slice(t0 + 128 * sub, t0 + 128 * sub + 128)
                        for ncol in range(2):
                            pj = ps.tile([128, 512], F32, tag="mm", bufs=2,
                                         name=f"pj{j}{sp}{sub}{ncol}")
                            for m2 in range(2):
                                nc.tensor.matmul(
                                    pj[:, :], yT_sb[m2][:, tsl],
                                    wp_sb[:, m2, 512 * ncol:512 * ncol + 512],
                                    start=(m2 == 0), stop=(m2 == 1))
                            dst = osb[:, sub, 512 * ncol:512 * ncol + 512]
                            if on_act:
                                nc.scalar.activation(dst, pj, AF.Copy)
                            else:
                                nc.vector.tensor_copy(dst, pj)
                            yield
                        nc.scalar.dma_start(out=out[tsl, :],
                                            in_=osb[:, sub, :])
                return go()

            def unit_tr(j, on_act=False):
                """Transpose yQ chunks of block j into yT via PE is_transpose
                (borrows an mm psum slot bitcast to bf16; the XBAR DMA route
                clogs the HWDGE queue). on_act: tail variant — psum->sbuf
                copies go to ACT, which is idle once the exps are done."""
                def go():
                    for qc in range(4 * j, 4 * j + 4):
                        tp = ps.tile([128, 512], F32, tag="mm", bufs=2,
                                     name=f"tp{qc}")
                        tpb = tp.bitcast(BF16)
                        for m in range(2):
                            nc.tensor.transpose(
                                tpb[:, 128 * m:128 * m + 128],
                                yQ_sb[:, qc, 128 * m:128 * m + 128], id_sb)
                        for m in range(2):
                            dst = yT_sb[m][:, 128 * qc:128 * qc + 128]
                            src = tpb[:, 128 * m:128 * m + 128]
                            if on_act:
                                nc.scalar.activation(dst, src, AF.Copy)
                            else:
                                nc.vector.tensor_copy(dst, src)
                        yield
                return go()

            def drain(gens):
                for g in gens:
                    for _ in g:
                        pass

            # prologue: block 0's q, then only the first 256 key-tokens of k
            # — enough for the first score pair; the second k half follows
            # while that pair's exp runs
            drain([unit_qk("q", 0, 0), unit_qk("q", 0, 1),
                   unit_qk("k", 0, 0, 0, 256), unit_qk("k", 0, 1, 0, 256)])
            k2nd = [unit_qk("k", 0, 0, 256, 256), unit_qk("k", 0, 1, 256, 256)]

            for j in range(4):
                if 2 <= j + 1 < 4:
                    dma_x(j + 1)
                npairs = 2 * (j + 1)
                # rolling filler queue (FIFO of generators); gens with
                # deadlines are tracked by name and force-completed in time
                vgens = [unit_v(j, si) for si in range(4)]
                queue = []
                if j > 0:
                    queue.append(unit_tr(j - 1))
                queue += vgens
                if j > 0:
                    queue += [unit_pj(j - 1, 0), unit_pj(j - 1, 1)]
                if j + 1 < 4:
                    queue += [unit_qk(w, j + 1, m)
                              for w in ("q", "k") for m in range(2)]
                nsteps = {0: 36, 1: 51, 2: 51, 3: 27}[j]
                rate = -(-nsteps // (HL * npairs))
                done = set()

                def pump(steps):
                    while steps > 0 and queue:
                        g = queue[0]
                        try:
                            next(g)
                            steps -= 1
                        except StopIteration:
                            done.add(g)
                            queue.pop(0)

                def force(gens):
                    for g in gens:
                        if g in done:
                            continue
                        for _ in g:
                            pass
                        done.add(g)
                        if g in queue:
                            queue.remove(g)

                def emit_st(h, p):
                    # pair p covers key chunks (2p, 2p+1)
                    hp = slice(32 * h, 32 * h + 32)
                    diag = p >= 2 * j
                    qs = 512 * j if p <= 2 * j else 512 * j + 256
                    n = 512 * j + 512 - qs
                    st = ps.tile([128, 2, 512], F32, tag="st", bufs=2,
                                 name=f"st{j}{h}{p}")
                    for c in range(2):
                        ki = 128 * (2 * p + c)
                        nc.tensor.matmul(st[:, c, 0:n],
                                         kT_sb[hp, :, ki:ki + 128],
                                         qT_sb[hp, :, qs:qs + n],
                                         start=True, stop=True,
                                         perf_mode=DR,
                                         tile_position=(32 * h, 0))
                    pt = ptp.tile([128, 2, 512], BF16, tag="pt",
                                  name=f"pt{j}{h}{p}")
                    nc.scalar.activation(pt[:, :, 0:n], st[:, :, 0:n],
                                         AF.Exp, scale=EXP_SCALE)
                    if diag:
                        # 0/1 masks post-exp, on Pool (keeps ACT/DVE free)
                        nc.gpsimd.tensor_tensor(
                            pt[:, 0, 0:128], pt[:, 0, 0:128], tri_sb,
                            op=OP.mult)
                        nc.gpsimd.tensor_tensor(
                            pt[:, 1, 0:256], pt[:, 1, 0:256], triw_sb,
                            op=OP.mult)
                    return pt, qs, n

                def emit_ot(h, p, yps, pt, qs, n):
                    # flipped att@V: pt stationary, v+ones moving. The four
                    # query-sub accumulation groups share one psum bank;
                    # start=True zeroes the WHOLE 2KB bank (ZERO_REGION_SIZE),
                    # so only the bank's first matmul sets it — the other
                    # groups' first writes land on pending-zero bytes and
                    # overwrite, then accumulate.
                    for qc in range(4):
                        for c in range(2):
                            i = 2 * p + c
                            if i > 4 * j + qc:
                                continue
                            off = 128 * qc + 512 * j - qs
                            if off < 0:
                                continue
                            nc.tensor.matmul(
                                yps[:, qc, 0:HD + 1],
                                pt[:, c, off:off + 128],
                                vO_sb[i][:, h, :],
                                start=(i == 0 and qc == 0),
                                stop=(i == 4 * j + qc),
                                skip_group_check=True)

                def emit_norm(h, yps):
                    # col 64 of each group is the denominator; token-major
                    # layout makes this a per-partition scalar multiply
                    rb = rcp.tile([128, 4], F32, tag=f"rb{h}", name=f"rb{j}{h}")
                    nc.vector.reciprocal(
                        rb, yps[:, :, HD:HD + 1].rearrange("p a b -> p (a b)"))
                    for qc in range(4):
                        nc.vector.tensor_scalar_mul(
                            yQ_sb[:, 4 * j + qc, 64 * h:64 * h + 64],
                            yps[:, qc, 0:HD], rb[:, qc:qc + 1])

                # one flat software-pipelined stream of (h, pair) across all
                # heads: no ACT bubble at h boundaries
                slots = [(h, p) for h in range(HL) for p in range(npairs)]
                LOOK = 2
                pts = {}
                yps_h = {}
                pts[0] = emit_st(*slots[0])
                if j == 0:  # second k half lands while pair 0's exp runs
                    drain(k2nd)
                pts[1] = emit_st(*slots[1])
                for idx, (h, p) in enumerate(slots):
                    if idx + LOOK < len(slots):
                        pts[idx + LOOK] = emit_st(*slots[idx + LOOK])
                    if h == 0:
                        if p == 2 * j:
                            force(vgens[:2])
                        elif p == 2 * j + 1:
                            force(vgens[2:])
                    if p == 0:
                        yps_h[h] = ps.tile([128, 4, 128], F32, tag="yps",
                                           bufs=2, name=f"yps{j}{h}")
                    emit_ot(h, p, yps_h[h], *pts.pop(idx))
                    if p == npairs - 1:
                        emit_norm(h, yps_h.pop(h))
                    if j > 0 or h > 0:  # j0-h0: x1 hasn't landed yet
                        pump(rate)
                pump(10 ** 9)  # flush fillers before the next j block
            drain([unit_tr(3), unit_pj(3, 0), unit_pj(3, 1)])

    _install_legalizer(nc)
    return nc


_NC_CACHE = None


def _get_nc():
    global _NC_CACHE
    if _NC_CACHE is None:
        _NC_CACHE = build_nc()
    return _NC_CACHE


# low/high 32-dim halves of each head -> planes (col 128c+32h+p of the
# permuted weight = dim 64h+32c+p of head h)
_PERM = np.array([64 * h + 32 * c + p
                  for c in range(2) for h in range(4) for p in range(32)])


def _comp8(a):
    """(fp8(a), fp8(a8/16), fp8(16*(a-a8))): one-psum compensated operands."""
    a = np.asarray(a, np.float32)
    a8 = a.astype(NP_FP8)
    a8f = a8.astype(np.float32)
    return a8, (a8f / 16.0).astype(NP_FP8), ((a - a8f) * 16.0).astype(NP_FP8)


def make_in_maps(x, Wq, bq, Wk, Wv, Wp):
    x = np.asarray(x, np.float32)
    xs = [_comp8(np.ascontiguousarray(x[b].T)) for b in range(2)]
    t01 = (np.arange(128)[None, :] >= np.arange(128)[:, None])
    tri01 = t01.astype(NP_BF16)
    triw01 = np.concatenate(
        [np.zeros((128, 128), NP_BF16), tri01], axis=1)
    Wq, Wk, Wv, Wp = (np.asarray(w, np.float32) for w in (Wq, Wk, Wv, Wp))
    bq = np.asarray(bq, np.float32)

    def wpack(wg):  # [DL, D] scaled -> [D, 3, DL] fp8 (w8 | w8/16 | 16*res)
        w8, wd, wr = _comp8(np.ascontiguousarray(wg.T))
        return np.ascontiguousarray(np.stack([w8, wd, wr], axis=1))

    in_maps = []
    for c in range(8):
        b, g = c // 4, c % 4
        sl = slice(DL * g, DL * g + DL)
        in_maps.append({
            "xT": xs[b][0],
            "xTd": xs[b][1],
            "xTr": xs[b][2],
            "wq": wpack(Wq[sl, :][_PERM, :] * WS),
            "wk": wpack(Wk[sl, :][_PERM, :] * WS),
            "wv": wpack(Wv[sl, :] * WS),
            "wp": np.ascontiguousarray(Wp[:, sl].T / WS).astype(NP_BF16),
            "bq": np.ascontiguousarray(bq[sl][_PERM]) * np.float32(WS),
            "tri": tri01,
            "triw": triw01,
            "ident": np.eye(128, dtype=np.float32).astype(NP_BF16),
        })
    return in_maps


def kernel(x, Wq, bq, Wk, bk, Wv, bv, Wp, bp, _run_kwargs=None):
    nc = _get_nc()
    in_maps = make_in_maps(x, Wq, bq, Wk, Wv, Wp)
    res = run_bass_kernel_spmd(nc, in_maps, list(range(8)), **(_run_kwargs or {}))
    corr = (np.asarray(bv, np.float32) @ np.asarray(Wp, np.float32).T
            + np.asarray(bp, np.float32))
    out = np.zeros((2, S, D), np.float32)
    for c in range(8):
        out[c // 4] += np.asarray(res.results[c]["out"], np.float32)
    out += corr[None, None, :]
    kernel.last_results = res
    return out


# revision 5
# speedup vs baseline: 1.0744x; 1.0289x over previous
"""Causal self-attention kernel for Trainium2, 8 NeuronCores — v2.

Problem: B=2, S=2048, D=1024, H=16 heads, Hd=64. fp32 in/out.
Sharding: core c -> batch b=c//4, head-group g=c%4 (4 heads, DL=256 dims).
Row-parallel output projection; host sums 4 partials per batch and adds the
(bv @ Wp.T + bp) correction row (softmax rows sum to 1; bk drops: softmax is
shift-invariant per query).

Speedups over the fp32r baseline (cost-model-guided; numerics-validated):
  - Scores matmul in fp8 DoubleRow: q/k stored x32-scaled fp8 in a
    [128, 2, S] layout whose planes are the low/high 32-dim halves of each
    head (host permutes W-q/k columns so the QKV psum lands directly in
    this layout) -> K=32x2 DoubleRow at 0.5 cyc/col, 2x the bf16 rate.
    fp8 q/k storage measured at 7.6e-3 rel err (gate 2e-2).
  - att@V flipped: P.T is the stationary operand, V the moving one, with a
    single ones-column emitting softmax denominators. The S.T-layout OT
    used only 64 of 128 stationary cols; the flip runs at 65 moving cols
    per 128x128 block = ~2x fewer PE cycles. y lands token-major, so
    normalization is a per-partition-scalar mul (cheap), and a DMA XBAR
    transpose returns y to dim-major for the projection.
  - exp batched per key-chunk PAIR: one ACT instruction covers [128, 2, n]
    across both psum banks of the pair's score tile, halving ACT's
    per-instruction overhead (ACT is the bottleneck engine).
  - causal masks applied post-exp as 0/1 bf16 multiplies on the Pool
    engine (otherwise idle), off ACT's critical path and off DVE.
  - x / v / P / y / Wp in bf16 (fp8 fails the error budget there);
    partial outputs leave as fp16 (halves output DMA; host sums in fp32).
"""
import json
import sys

sys.path.insert(0, "/opt/trn_rl_repo")

import numpy as np
import ml_dtypes

import concourse.bass as bass
import concourse.mybir as mybir
import concourse.tile as tile
from concourse.bass_utils import run_bass_kernel_spmd

F32 = mybir.dt.float32
F16 = mybir.dt.float16
BF16 = mybir.dt.bfloat16
FP8 = mybir.dt.float8e4
NP_FP8 = ml_dtypes.float8_e4m3
NP_BF16 = ml_dtypes.bfloat16
AF = mybir.ActivationFunctionType
OP = mybir.AluOpType
DR = mybir.MatmulPerfMode.DoubleRow

S = 2048          # tokens per batch (= per core)
D = 1024          # model dim
HL = 4            # heads per core
HD = 64           # head dim
DL = HL * HD      # local dims per core (256)
WS = 32.0         # fp8 q/k scale (scores x1024 -> exp scale 2^-13)
EXP_SCALE = 0.125 / (WS * WS)


def _legalize_waits_json(bir_bytes: bytes) -> bytes:
    """walrus allows <=1 sem-wait per instruction (<=2 for EventSemaphore);
    spill extras onto EventSemaphore instructions."""
    j = json.loads(bir_bytes)
    for fn in j["functions"]:
        for bb in fn["blocks"]:
            out = []
            for inst in bb["instructions"]:
                si = inst.get("sync_info") or {}
                ws = si.get("on_wait") or []
                cap = 2 if inst.get("opcode") == "EventSemaphore" else 1
                if len(ws) > cap:
                    extras, keep = ws[:-cap], ws[-cap:]
                    k = 0
                    while extras:
                        chunk, extras = extras[:2], extras[2:]
                        out.append({
                            "debug": inst.get("debug", 0),
                            "engine": inst["engine"],
                            "ins": [],
                            "name": f"{inst['name']}_wfix{k}",
                            "opcode": "EventSemaphore",
                            "outs": [],
                            "sync_info": {"on_update": [], "on_wait": chunk},
                        })
                        k += 1
                    si["on_wait"] = keep
                out.append(inst)
            bb["instructions"] = out
    return json.dumps(j).encode()


def _install_legalizer(nc):
    orig = nc.to_json_bytes
    nc.to_json_bytes = lambda: _legalize_waits_json(orig())


def build_nc() -> bass.Bass:
    nc = bass.Bass(trn_type="TRN2", num_devices=8)

    # compensated fp8 QKV: projections accumulate x8@w8 + xr16@(w8/16) +
    # (x8/16)@wr16 in ONE psum group (the 1/16 scales are folded into
    # pre-scaled fp8 operand copies), 12 DoubleRow matmuls at 2x bf16 rate
    xT = nc.dram_tensor("xT", [D, S], FP8, kind="ExternalInput")      # x8
    xTr = nc.dram_tensor("xTr", [D, S], FP8, kind="ExternalInput")    # 16*res
    xTd = nc.dram_tensor("xTd", [D, S], FP8, kind="ExternalInput")    # x8/16
    wq = nc.dram_tensor("wq", [D, 3, DL], FP8, kind="ExternalInput")  # perm'd
    wk = nc.dram_tensor("wk", [D, 3, DL], FP8, kind="ExternalInput")  # perm'd
    wv = nc.dram_tensor("wv", [D, 3, DL], FP8, kind="ExternalInput")
    wp = nc.dram_tensor("wp", [DL, D], BF16, kind="ExternalInput")    # Wp.T/32
    bq = nc.dram_tensor("bq", [DL], F32, kind="ExternalInput")        # 32bq
    tri = nc.dram_tensor("tri", [128, 128], BF16, kind="ExternalInput")
    triw = nc.dram_tensor("triw", [128, 256], BF16, kind="ExternalInput")
    ident = nc.dram_tensor("ident", [128, 128], BF16, kind="ExternalInput")
    out = nc.dram_tensor("out", [S, D], F16, kind="ExternalOutput")

    with tile.TileContext(nc) as tc:
        with tc.tile_pool(name="const", bufs=1) as const, \
             tc.tile_pool(name="acts", bufs=1) as acts, \
             tc.tile_pool(name="xin", bufs=2) as xpool, \
             tc.tile_pool(name="pt", bufs=4) as ptp, \
             tc.tile_pool(name="rc", bufs=1) as rcp, \
             tc.tile_pool(name="outp", bufs=3) as outp, \
             tc.tile_pool(name="ps", bufs=1, space="PSUM") as ps:
            wq_sb = const.tile([128, 8, 3, DL], FP8)
            wk_sb = const.tile([128, 8, 3, DL], FP8)
            wv_sb = const.tile([128, 8, 3, DL], FP8)
            wp_sb = const.tile([128, 2, D], BF16)
            bq_sb = const.tile([128, 2], F32)
            tri_sb = const.tile([128, 128], BF16)     # 0/1 causal triangle
            triw_sb = const.tile([128, 256], BF16)    # [zeros | triangle]
            id_sb = const.tile([128, 128], BF16)      # PE-transpose identity

            # q/k: [128, 2, S] fp8; partition 32h+p, plane c = head h's
            # dim 32c+p (weight columns host-permuted to produce this)
            qT_sb = acts.tile([128, 2, S], FP8, name="qT")
            kT_sb = acts.tile([128, 2, S], FP8, name="kT")
            # y token-major: [128 tok, 16 chunks, 256 dims] bf16
            yQ_sb = acts.tile([128, 16, DL], BF16, name="yQ")
            # y dim-major (post DMA-transpose) for the projection
            yT_sb = [acts.tile([128, S], BF16, name=f"yT{m}") for m in range(2)]
            # v per key chunk: [128 keys, 4 heads, 64 v + 1 one]
            vO_sb = [acts.tile([128, HL, HD + 1], BF16, name=f"vO{i}")
                     for i in range(16)]

            x3 = xT[:].rearrange("(kc p) t -> p kc t", p=128)
            xr3 = xTr[:].rearrange("(kc p) t -> p kc t", p=128)
            xd3 = xTd[:].rearrange("(kc p) t -> p kc t", p=128)
            wq3 = wq[:].rearrange("(kc p) r m -> p kc r m", p=128)
            wk3 = wk[:].rearrange("(kc p) r m -> p kc r m", p=128)
            wv3 = wv[:].rearrange("(kc p) r m -> p kc r m", p=128)

            xsb_tiles = {}

            def dma_x(t4):
                xsb = xpool.tile([128, 8, 512], FP8, tag="x8", name=f"x{t4}")
                xrb = xpool.tile([128, 8, 512], FP8, tag="xr", name=f"xr{t4}")
                xdb = xpool.tile([128, 8, 512], FP8, tag="xd", name=f"xd{t4}")
                xsb_tiles[t4] = (xsb, xrb, xdb)
                ts = slice(512 * t4, 512 * t4 + 512)
                nc.sync.dma_start(out=xsb, in_=x3[:, :, ts])
                nc.sync.dma_start(out=xrb, in_=xr3[:, :, ts])
                nc.sync.dma_start(out=xdb, in_=xd3[:, :, ts])
                return xsb

            # startup: few LARGE DMAs (the HWDGE queue costs ~600ns per DMA
            # instruction, so many small transfers serialize the prologue).
            # Critical set for the first scores: x0 + full wq/wk (every head
            # reads both dim-half planes); wv next (first att@V), then x1.
            xsb0 = xpool.tile([128, 8, 512], FP8, tag="x8", name="x0")
            xrb0 = xpool.tile([128, 8, 512], FP8, tag="xr", name="xr0")
            xdb0 = xpool.tile([128, 8, 512], FP8, tag="xd", name="xd0")
            xsb_tiles[0] = (xsb0, xrb0, xdb0)
            # big input streams on the SP hwdge queue; tiny constants go out
            # on the Activation hwdge queue in parallel (each dma_start costs
            # ~650ns of issue time on its sequencer)
            nc.sync.dma_start(out=id_sb, in_=ident[:])  # first: PE warmup
            nc.scalar.dma_start(out=bq_sb,
                                in_=bq[:].rearrange("(m p) -> p m", p=128))
            nc.scalar.dma_start(out=tri_sb, in_=tri[:])
            nc.scalar.dma_start(out=triw_sb, in_=triw[:])
            nc.sync.dma_start(out=xsb0, in_=x3[:, :, 0:512])
            nc.sync.dma_start(out=wq_sb, in_=wq3)
            nc.sync.dma_start(out=xrb0, in_=xr3[:, :, 0:512])
            nc.sync.dma_start(out=xdb0, in_=xd3[:, :, 0:512])
            nc.sync.dma_start(out=wk_sb, in_=wk3)
            nc.sync.dma_start(out=wv_sb, in_=wv3)
            dma_x(1)
            nc.sync.dma_start(
                out=wp_sb, in_=wp[:].rearrange("(m p) n -> p m n", p=128))
            for i in range(16):
                nc.gpsimd.memset(vO_sb[i][:, :, HD:HD + 1], 1.0)

            # p-state warmup: the PE ramps 0.65 -> 1.2 -> 2.4 GHz over 3us of
            # CONTINUOUS execution. Keep it spinning on the identity tile
            # while the x0/wq/wk streams land so the real QKV runs at full
            # clock. (~70 x 128-col transposes span the DMA window.)
            # preload the ACT Exp table during the DMA wait: the first real
            # exp otherwise pays the 1283ns table load on the critical path
            scr = const.tile([128, 1], F32)
            nc.vector.memset(scr, 0.0)
            nc.scalar.activation(scr, scr, AF.Exp, scale=0.0)

            # (pe_busy_start pins at the FIRST PE activity and never resets,
            # so a few early matmuls suffice to have everything 3us+ later
            # run at 2.4GHz)
            warm = ps.tile([128, 4, 128], F32, tag="yps", bufs=2, name="warm")
            wb = warm.bitcast(BF16)
            for i in range(40):
                nc.tensor.transpose(wb[:, 0, 0:128], id_sb, id_sb)

            # --- filler units as GENERATORS: yield between small batches of
            # PE work so the scheduler can trickle them between score pairs
            # without stalling the ST->exp pipeline (ACT is the bottleneck
            # engine; it must never wait on a long filler burst) ---
            def unit_qk(which, t4, m, t0=0, tn=512):
                wsb = wq_sb if which == "q" else wk_sb
                dst = qT_sb if which == "q" else kT_sb

                def go():
                    tsl = slice(512 * t4 + t0, 512 * t4 + t0 + tn)
                    x8, xr, xd = xsb_tiles[t4]
                    dsl = slice(128 * m, 128 * m + 128)
                    p = ps.tile([128, 512], F32, tag="mm", bufs=2,
                                name=f"{which}{t4}{m}{t0}")
                    for r, xop in ((0, x8), (1, xr), (2, xd)):
                        for kp in range(4):
                            kk = slice(2 * kp, 2 * kp + 2)
                            nc.tensor.matmul(p[:, 0:tn], wsb[:, kk, r, dsl],
                                             xop[:, kk, t0:t0 + tn],
                                             start=(r == 0 and kp == 0),
                                             stop=(r == 2 and kp == 3),
                                             perf_mode=DR)
                            if kp % 2 == 1 and (r, kp) != (2, 3):
                                yield
                    if which == "q":
                        nc.vector.tensor_scalar_add(dst[:, m, tsl], p[:, 0:tn],
                                                    bq_sb[:, m:m + 1])
                    else:
                        nc.vector.tensor_copy(dst[:, m, tsl], p[:, 0:tn])
                return go()

            def unit_v(t4, si):
                def go():
                    x8, xr, xd = xsb_tiles[t4]
                    tl = slice(128 * si, 128 * si + 128)
                    p = ps.tile([128, DL], F32, tag="mm", bufs=2,
                                name=f"v{t4}{si}")
                    for r, xop in ((0, x8), (1, xr), (2, xd)):
                        for kp in range(4):
                            kk = slice(2 * kp, 2 * kp + 2)
                            nc.tensor.matmul(p[:, :], xop[:, kk, tl],
                                             wv_sb[:, kk, r, :],
                                             start=(r == 0 and kp == 0),
                                             stop=(r == 2 and kp == 3),
                                             perf_mode=DR)
                        if r < 2:
                            yield
                    v3 = vO_sb[4 * t4 + si]
                    nc.vector.tensor_copy(
                        v3[:, :, 0:HD], p[:].rearrange("p (h c) -> p h c", h=HL))
                return go()

            def unit_pj(j, sp, on_act=False):
                """Project 256 tokens; fp16 partials out. on_act: route the
                psum->sbuf copies to ACT (for the tail, when exps are done)."""
                def go():
                    t0 = 512 * j + 256 * sp
                    osb = outp.tile([128, 2, D], F16, tag="o", name=f"o{j}{sp}")
                    for sub in range(2):
                        tsl = slice(t0 + 128 * sub, t0 + 128 * sub + 128)
                        for ncol in range(2):
                            pj = ps.tile([128, 512], F32, tag="mm", bufs=2,
                                         name=f"pj{j}{sp}{sub}{ncol}")
                            for m2 in range(2):
                                nc.tensor.matmul(
                                    pj[:, :], yT_sb[m2][:, tsl],
                                    wp_sb[:, m2, 512 * ncol:512 * ncol + 512],
                                    start=(m2 == 0), stop=(m2 == 1))
                            dst = osb[:, sub, 512 * ncol:512 * ncol + 512]
                            if on_act:
                                nc.scalar.activation(dst, pj, AF.Copy)
                            else:
                                nc.vector.tensor_copy(dst, pj)
                            yield
                        nc.scalar.dma_start(out=out[tsl, :],
                                            in_=osb[:, sub, :])
                return go()

            def unit_tr(j, on_act=False):
                """Transpose yQ chunks of block j into yT via PE is_transpose
                (borrows an mm psum slot bitcast to bf16; the XBAR DMA route
                clogs the HWDGE queue). on_act: tail variant — psum->sbuf
                copies go to ACT, which is idle once the exps are done."""
                def go():
                    for qc in range(4 * j, 4 * j + 4):
                        tp = ps.tile([128, 512], F32, tag="mm", bufs=2,
                                     name=f"tp{qc}")
                        tpb = tp.bitcast(BF16)
                        for m in range(2):
                            nc.tensor.transpose(
                                tpb[:, 128 * m:128 * m + 128],
                                yQ_sb[:, qc, 128 * m:128 * m + 128], id_sb)
                        for m in range(2):
                            dst = yT_sb[m][:, 128 * qc:128 * qc + 128]
                            src = tpb[:, 128 * m:128 * m + 128]
                            if on_act:
                                nc.scalar.activation(dst, src, AF.Copy)
                            else:
                                nc.vector.tensor_copy(dst, src)
                        yield
                return go()

            def drain(gens):
                for g in gens:
                    for _ in g:
                        pass

            # prologue: block 0's q, then only the first 256 key-tokens of k
            # — enough for the first score pair; the second k half follows
            # while that pair's exp runs
            drain([unit_qk("q", 0, 0), unit_qk("q", 0, 1),
                   unit_qk("k", 0, 0, 0, 256), unit_qk("k", 0, 1, 0, 256)])
            k2nd = [unit_qk("k", 0, 0, 256, 256), unit_qk("k", 0, 1, 256, 256)]

            for j in range(4):
                if 2 <= j + 1 < 4:
                    dma_x(j + 1)
                npairs = 2 * (j + 1)
                # rolling filler queue (FIFO of generators); gens with
                # deadlines are tracked by name and force-completed in time
                vgens = [unit_v(j, si) for si in range(4)]
                queue = []
                if j > 0:
                    queue.append(unit_tr(j - 1))
                queue += vgens
                if j > 0:
                    queue += [unit_pj(j - 1, 0), unit_pj(j - 1, 1)]
                if j + 1 < 4:
                    queue += [unit_qk(w, j + 1, m)
                              for w in ("q", "k") for m in range(2)]
                nsteps = {0: 36, 1: 32, 2: 24, 3: 27}[j]
                rate = -(-nsteps // (HL * npairs))
                done = set()

                def pump(steps):
                    while steps > 0 and queue:
                        g = queue[0]
                        try:
                            next(g)
                            steps -= 1
                        except StopIteration:
                            done.add(g)
                            queue.pop(0)

                def force(gens):
                    for g in gens:
                        if g in done:
                            continue
                        for _ in g:
                            pass
                        done.add(g)
                        if g in queue:
                            queue.remove(g)

                def emit_st(h, p):
                    # pair p covers key chunks (2p, 2p+1)
                    hp = slice(32 * h, 32 * h + 32)
                    diag = p >= 2 * j
                    qs = 512 * j if p <= 2 * j else 512 * j + 256
                    n = 512 * j + 512 - qs
                    st = ps.tile([128, 2, 512], F32, tag="st", bufs=2,
                                 name=f"st{j}{h}{p}")
                    for c in range(2):
                        ki = 128 * (2 * p + c)
                        nc.tensor.matmul(st[:, c, 0:n],
                                         kT_sb[hp, :, ki:ki + 128],
                                         qT_sb[hp, :, qs:qs + n],
                                         start=True, stop=True,
                                         perf_mode=DR,
                                         tile_position=(32 * h, 0))
                    pt = ptp.tile([128, 2, 512], BF16, tag="pt",
                                  name=f"pt{j}{h}{p}")
                    nc.scalar.activation(pt[:, :, 0:n], st[:, :, 0:n],
                                         AF.Exp, scale=EXP_SCALE)
                    if diag:
                        # 0/1 masks post-exp, on Pool (keeps ACT/DVE free)
                        nc.gpsimd.tensor_tensor(
                            pt[:, 0, 0:128], pt[:, 0, 0:128], tri_sb,
                            op=OP.mult)
                        nc.gpsimd.tensor_tensor(
                            pt[:, 1, 0:256], pt[:, 1, 0:256], triw_sb,
                            op=OP.mult)
                    return pt, qs, n

                def emit_ot(h, p, yps, pt, qs, n):
                    # flipped att@V: pt stationary, v+ones moving. The four
                    # query-sub accumulation groups share one psum bank;
                    # start=True zeroes the WHOLE 2KB bank (ZERO_REGION_SIZE),
                    # so only the bank's first matmul sets it — the other
                    # groups' first writes land on pending-zero bytes and
                    # overwrite, then accumulate.
                    for qc in range(4):
                        for c in range(2):
                            i = 2 * p + c
                            if i > 4 * j + qc:
                                continue
                            off = 128 * qc + 512 * j - qs
                            if off < 0:
                                continue
                            nc.tensor.matmul(
                                yps[:, qc, 0:HD + 1],
                                pt[:, c, off:off + 128],
                                vO_sb[i][:, h, :],
                                start=(i == 0 and qc == 0),
                                stop=(i == 4 * j + qc),
                                skip_group_check=True)

                def emit_norm(h, yps):
                    # col 64 of each group is the denominator; token-major
                    # layout makes this a per-partition scalar multiply
                    rb = rcp.tile([128, 4], F32, tag=f"rb{h}", name=f"rb{j}{h}")
                    nc.vector.reciprocal(
                        rb, yps[:, :, HD:HD + 1].rearrange("p a b -> p (a b)"))
                    for qc in range(4):
                        nc.vector.tensor_scalar_mul(
                            yQ_sb[:, 4 * j + qc, 64 * h:64 * h + 64],
                            yps[:, qc, 0:HD], rb[:, qc:qc + 1])

                # one flat software-pipelined stream of (h, pair) across all
                # heads: no ACT bubble at h boundaries
                slots = [(h, p) for h in range(HL) for p in range(npairs)]
                LOOK = 2
                pts = {}
                yps_h = {}
                pts[0] = emit_st(*slots[0])
                if j == 0:  # second k half lands while pair 0's exp runs
                    drain(k2nd)
                pts[1] = emit_st(*slots[1])
                for idx, (h, p) in enumerate(slots):
                    if idx + LOOK < len(slots):
                        pts[idx + LOOK] = emit_st(*slots[idx + LOOK])
                    if h == 0:
                        if p == 2 * j:
                            force(vgens[:2])
                        elif p == 2 * j + 1:
                            force(vgens[2:])
                    if p == 0:
                        yps_h[h] = ps.tile([128, 4, 128], F32, tag="yps",
                                           bufs=2, name=f"yps{j}{h}")
                    emit_ot(h, p, yps_h[h], *pts.pop(idx))
                    if p == npairs - 1:
                        emit_norm(h, yps_h.pop(h))
                    if j > 0 or h > 0:  # j0-h0: x1 hasn't landed yet
                        pump(rate)
                pump(10 ** 9)  # flush fillers before the next j block
            drain([unit_tr(3), unit_pj(3, 0), unit_pj(3, 1)])

    _install_legalizer(nc)
    return nc


_NC_CACHE = None


def _get_nc():
    global _NC_CACHE
    if _NC_CACHE is None:
        _NC_CACHE = build_nc()
    return _NC_CACHE


# low/high 32-dim halves of each head -> planes (col 128c+32h+p of the
# permuted weight = dim 64h+32c+p of head h)
_PERM = np.array([64 * h + 32 * c + p
                  for c in range(2) for h in range(4) for p in range(32)])


def _comp8(a):
    """(fp8(a), fp8(a8/16), fp8(16*(a-a8))): one-psum compensated operands."""
    a = np.asarray(a, np.float32)
    a8 = a.astype(NP_FP8)
    a8f = a8.astype(np.float32)
    return a8, (a8f / 16.0).astype(NP_FP8), ((a - a8f) * 16.0).astype(NP_FP8)


def make_in_maps(x, Wq, bq, Wk, Wv, Wp):
    x = np.asarray(x, np.float32)
    xs = [_comp8(np.ascontiguousarray(x[b].T)) for b in range(2)]
    t01 = (np.arange(128)[None, :] >= np.arange(128)[:, None])
    tri01 = t01.astype(NP_BF16)
    triw01 = np.concatenate(
        [np.zeros((128, 128), NP_BF16), tri01], axis=1)
    Wq, Wk, Wv, Wp = (np.asarray(w, np.float32) for w in (Wq, Wk, Wv, Wp))
    bq = np.asarray(bq, np.float32)

    def wpack(wg):  # [DL, D] scaled -> [D, 3, DL] fp8 (w8 | w8/16 | 16*res)
        w8, wd, wr = _comp8(np.ascontiguousarray(wg.T))
        return np.ascontiguousarray(np.stack([w8, wd, wr], axis=1))

    in_maps = []
    for c in range(8):
        b, g = c // 4, c % 4
        sl = slice(DL * g, DL * g + DL)
        in_maps.append({
            "xT": xs[b][0],
            "xTd": xs[b][1],
            "xTr": xs[b][2],
            "wq": wpack(Wq[sl, :][_PERM, :] * WS),
            "wk": wpack(Wk[sl, :][_PERM, :] * WS),
            "wv": wpack(Wv[sl, :] * WS),
            "wp": np.ascontiguousarray(Wp[:, sl].T / WS).astype(NP_BF16),
            "bq": np.ascontiguousarray(bq[sl][_PERM]) * np.float32(WS),
            "tri": tri01,
            "triw": triw01,
            "ident": np.eye(128, dtype=np.float32).astype(NP_BF16),
        })
    return in_maps


def kernel(x, Wq, bq, Wk, bk, Wv, bv, Wp, bp, _run_kwargs=None):
    nc = _get_nc()
    in_maps = make_in_maps(x, Wq, bq, Wk, Wv, Wp)
    res = run_bass_kernel_spmd(nc, in_maps, list(range(8)), **(_run_kwargs or {}))
    corr = (np.asarray(bv, np.float32) @ np.asarray(Wp, np.float32).T
            + np.asarray(bp, np.float32))
    out = np.zeros((2, S, D), np.float32)
    for c in range(8):
        out[c // 4] += np.asarray(res.results[c]["out"], np.float32)
    out += corr[None, None, :]
    kernel.last_results = res
    return out


# revision 6
# speedup vs baseline: 1.0792x; 1.0044x over previous
"""Causal self-attention kernel for Trainium2, 8 NeuronCores — v2.

Problem: B=2, S=2048, D=1024, H=16 heads, Hd=64. fp32 in/out.
Sharding: core c -> batch b=c//4, head-group g=c%4 (4 heads, DL=256 dims).
Row-parallel output projection; host sums 4 partials per batch and adds the
(bv @ Wp.T + bp) correction row (softmax rows sum to 1; bk drops: softmax is
shift-invariant per query).

Speedups over the fp32r baseline (cost-model-guided; numerics-validated):
  - Scores matmul in fp8 DoubleRow: q/k stored x32-scaled fp8 in a
    [128, 2, S] layout whose planes are the low/high 32-dim halves of each
    head (host permutes W-q/k columns so the QKV psum lands directly in
    this layout) -> K=32x2 DoubleRow at 0.5 cyc/col, 2x the bf16 rate.
    fp8 q/k storage measured at 7.6e-3 rel err (gate 2e-2).
  - att@V flipped: P.T is the stationary operand, V the moving one, with a
    single ones-column emitting softmax denominators. The S.T-layout OT
    used only 64 of 128 stationary cols; the flip runs at 65 moving cols
    per 128x128 block = ~2x fewer PE cycles. y lands token-major, so
    normalization is a per-partition-scalar mul (cheap), and a DMA XBAR
    transpose returns y to dim-major for the projection.
  - exp batched per key-chunk PAIR: one ACT instruction covers [128, 2, n]
    across both psum banks of the pair's score tile, halving ACT's
    per-instruction overhead (ACT is the bottleneck engine).
  - causal masks applied post-exp as 0/1 bf16 multiplies on the Pool
    engine (otherwise idle), off ACT's critical path and off DVE.
  - x / v / P / y / Wp in bf16 (fp8 fails the error budget there);
    partial outputs leave as fp16 (halves output DMA; host sums in fp32).
"""
import json
import sys

sys.path.insert(0, "/opt/trn_rl_repo")

import numpy as np
import ml_dtypes

import concourse.bass as bass
import concourse.mybir as mybir
import concourse.tile as tile
from concourse.bass_utils import run_bass_kernel_spmd

F32 = mybir.dt.float32
F16 = mybir.dt.float16
BF16 = mybir.dt.bfloat16
FP8 = mybir.dt.float8e4
NP_FP8 = ml_dtypes.float8_e4m3
NP_BF16 = ml_dtypes.bfloat16
AF = mybir.ActivationFunctionType
OP = mybir.AluOpType
DR = mybir.MatmulPerfMode.DoubleRow

S = 2048          # tokens per batch (= per core)
D = 1024          # model dim
HL = 4            # heads per core
HD = 64           # head dim
DL = HL * HD      # local dims per core (256)
WS = 32.0         # fp8 q/k scale (scores x1024 -> exp scale 2^-13)
EXP_SCALE = 0.125 / (WS * WS)


def _legalize_waits_json(bir_bytes: bytes) -> bytes:
    """walrus allows <=1 sem-wait per instruction (<=2 for EventSemaphore);
    spill extras onto EventSemaphore instructions."""
    j = json.loads(bir_bytes)
    for fn in j["functions"]:
        for bb in fn["blocks"]:
            out = []
            for inst in bb["instructions"]:
                si = inst.get("sync_info") or {}
                ws = si.get("on_wait") or []
                cap = 2 if inst.get("opcode") == "EventSemaphore" else 1
                if len(ws) > cap:
                    extras, keep = ws[:-cap], ws[-cap:]
                    k = 0
                    while extras:
                        chunk, extras = extras[:2], extras[2:]
                        out.append({
                            "debug": inst.get("debug", 0),
                            "engine": inst["engine"],
                            "ins": [],
                            "name": f"{inst['name']}_wfix{k}",
                            "opcode": "EventSemaphore",
                            "outs": [],
                            "sync_info": {"on_update": [], "on_wait": chunk},
                        })
                        k += 1
                    si["on_wait"] = keep
                out.append(inst)
            bb["instructions"] = out
    return json.dumps(j).encode()


def _install_legalizer(nc):
    orig = nc.to_json_bytes
    nc.to_json_bytes = lambda: _legalize_waits_json(orig())


def build_nc() -> bass.Bass:
    nc = bass.Bass(trn_type="TRN2", num_devices=8)

    # compensated fp8 QKV: projections accumulate x8@w8 + xr16@(w8/16) +
    # (x8/16)@wr16 in ONE psum group (the 1/16 scales are folded into
    # pre-scaled fp8 operand copies), 12 DoubleRow matmuls at 2x bf16 rate
    xT = nc.dram_tensor("xT", [D, S], FP8, kind="ExternalInput")      # x8
    xTr = nc.dram_tensor("xTr", [D, S], FP8, kind="ExternalInput")    # 16*res
    xTd = nc.dram_tensor("xTd", [D, S], FP8, kind="ExternalInput")    # x8/16
    wq = nc.dram_tensor("wq", [D, 3, DL], FP8, kind="ExternalInput")  # perm'd
    wk = nc.dram_tensor("wk", [D, 3, DL], FP8, kind="ExternalInput")  # perm'd
    wv = nc.dram_tensor("wv", [D, 3, DL], FP8, kind="ExternalInput")
    wp = nc.dram_tensor("wp", [DL, D], BF16, kind="ExternalInput")    # Wp.T/32
    bq = nc.dram_tensor("bq", [DL], F32, kind="ExternalInput")        # 32bq
    tri = nc.dram_tensor("tri", [128, 128], BF16, kind="ExternalInput")
    triw = nc.dram_tensor("triw", [128, 256], BF16, kind="ExternalInput")
    ident = nc.dram_tensor("ident", [128, 128], BF16, kind="ExternalInput")
    out = nc.dram_tensor("out", [S, D], F16, kind="ExternalOutput")

    with tile.TileContext(nc) as tc:
        with tc.tile_pool(name="const", bufs=1) as const, \
             tc.tile_pool(name="acts", bufs=1) as acts, \
             tc.tile_pool(name="xin", bufs=2) as xpool, \
             tc.tile_pool(name="pt", bufs=4) as ptp, \
             tc.tile_pool(name="rc", bufs=1) as rcp, \
             tc.tile_pool(name="outp", bufs=3) as outp, \
             tc.tile_pool(name="ps", bufs=1, space="PSUM") as ps:
            wq_sb = const.tile([128, 8, 3, DL], FP8)
            wk_sb = const.tile([128, 8, 3, DL], FP8)
            wv_sb = const.tile([128, 8, 3, DL], FP8)
            wp_sb = const.tile([128, 2, D], BF16)
            bq_sb = const.tile([128, 2], F32)
            tri_sb = const.tile([128, 128], BF16)     # 0/1 causal triangle
            triw_sb = const.tile([128, 256], BF16)    # [zeros | triangle]
            id_sb = const.tile([128, 128], BF16)      # PE-transpose identity

            # q/k: [128, 2, S] fp8; partition 32h+p, plane c = head h's
            # dim 32c+p (weight columns host-permuted to produce this)
            qT_sb = acts.tile([128, 2, S], FP8, name="qT")
            kT_sb = acts.tile([128, 2, S], FP8, name="kT")
            # y token-major: [128 tok, 16 chunks, 256 dims] bf16
            yQ_sb = acts.tile([128, 16, DL], BF16, name="yQ")
            # y dim-major (post DMA-transpose) for the projection
            yT_sb = [acts.tile([128, S], BF16, name=f"yT{m}") for m in range(2)]
            # v per key chunk: [128 keys, 4 heads, 64 v + 1 one]
            vO_sb = [acts.tile([128, HL, HD + 1], BF16, name=f"vO{i}")
                     for i in range(16)]

            x3 = xT[:].rearrange("(kc p) t -> p kc t", p=128)
            xr3 = xTr[:].rearrange("(kc p) t -> p kc t", p=128)
            xd3 = xTd[:].rearrange("(kc p) t -> p kc t", p=128)
            wq3 = wq[:].rearrange("(kc p) r m -> p kc r m", p=128)
            wk3 = wk[:].rearrange("(kc p) r m -> p kc r m", p=128)
            wv3 = wv[:].rearrange("(kc p) r m -> p kc r m", p=128)

            xsb_tiles = {}

            def dma_x(t4):
                xsb = xpool.tile([128, 8, 512], FP8, tag="x8", name=f"x{t4}")
                xrb = xpool.tile([128, 8, 512], FP8, tag="xr", name=f"xr{t4}")
                xdb = xpool.tile([128, 8, 512], FP8, tag="xd", name=f"xd{t4}")
                xsb_tiles[t4] = (xsb, xrb, xdb)
                ts = slice(512 * t4, 512 * t4 + 512)
                nc.sync.dma_start(out=xsb, in_=x3[:, :, ts])
                nc.sync.dma_start(out=xrb, in_=xr3[:, :, ts])
                nc.sync.dma_start(out=xdb, in_=xd3[:, :, ts])
                return xsb

            # startup: few LARGE DMAs (the HWDGE queue costs ~600ns per DMA
            # instruction, so many small transfers serialize the prologue).
            # Critical set for the first scores: x0 + full wq/wk (every head
            # reads both dim-half planes); wv next (first att@V), then x1.
            xsb0 = xpool.tile([128, 8, 512], FP8, tag="x8", name="x0")
            xrb0 = xpool.tile([128, 8, 512], FP8, tag="xr", name="xr0")
            xdb0 = xpool.tile([128, 8, 512], FP8, tag="xd", name="xd0")
            xsb_tiles[0] = (xsb0, xrb0, xdb0)
            # big input streams on the SP hwdge queue; tiny constants go out
            # on the Activation hwdge queue in parallel (each dma_start costs
            # ~650ns of issue time on its sequencer)
            nc.sync.dma_start(out=id_sb, in_=ident[:])  # first: PE warmup
            nc.scalar.dma_start(out=bq_sb,
                                in_=bq[:].rearrange("(m p) -> p m", p=128))
            nc.scalar.dma_start(out=tri_sb, in_=tri[:])
            nc.scalar.dma_start(out=triw_sb, in_=triw[:])
            nc.sync.dma_start(out=xsb0, in_=x3[:, :, 0:512])
            nc.sync.dma_start(out=wq_sb, in_=wq3)
            nc.sync.dma_start(out=xrb0, in_=xr3[:, :, 0:512])
            nc.sync.dma_start(out=xdb0, in_=xd3[:, :, 0:512])
            nc.sync.dma_start(out=wk_sb, in_=wk3)
            nc.sync.dma_start(out=wv_sb, in_=wv3)
            dma_x(1)
            nc.sync.dma_start(
                out=wp_sb, in_=wp[:].rearrange("(m p) n -> p m n", p=128))
            for i in range(16):
                nc.gpsimd.memset(vO_sb[i][:, :, HD:HD + 1], 1.0)

            # p-state warmup: the PE ramps 0.65 -> 1.2 -> 2.4 GHz over 3us of
            # CONTINUOUS execution. Keep it spinning on the identity tile
            # while the x0/wq/wk streams land so the real QKV runs at full
            # clock. (~70 x 128-col transposes span the DMA window.)
            # preload the ACT Exp table during the DMA wait: the first real
            # exp otherwise pays the 1283ns table load on the critical path
            scr = const.tile([128, 1], F32)
            nc.vector.memset(scr, 0.0)
            nc.scalar.activation(scr, scr, AF.Exp, scale=0.0)

            # (pe_busy_start pins at the FIRST PE activity and never resets,
            # so a few early matmuls suffice to have everything 3us+ later
            # run at 2.4GHz)
            warm = ps.tile([128, 4, 128], F32, tag="yps", bufs=2, name="warm")
            wb = warm.bitcast(BF16)
            for i in range(40):
                nc.tensor.transpose(wb[:, 0, 0:128], id_sb, id_sb)

            # --- filler units as GENERATORS: yield between small batches of
            # PE work so the scheduler can trickle them between score pairs
            # without stalling the ST->exp pipeline (ACT is the bottleneck
            # engine; it must never wait on a long filler burst) ---
            def unit_qk(which, t4, m, t0=0, tn=512):
                wsb = wq_sb if which == "q" else wk_sb
                dst = qT_sb if which == "q" else kT_sb

                def go():
                    tsl = slice(512 * t4 + t0, 512 * t4 + t0 + tn)
                    x8, xr, xd = xsb_tiles[t4]
                    dsl = slice(128 * m, 128 * m + 128)
                    p = ps.tile([128, 512], F32, tag="mm", bufs=2,
                                name=f"{which}{t4}{m}{t0}")
                    for r, xop in ((0, x8), (1, xr), (2, xd)):
                        for kp in range(4):
                            kk = slice(2 * kp, 2 * kp + 2)
                            nc.tensor.matmul(p[:, 0:tn], wsb[:, kk, r, dsl],
                                             xop[:, kk, t0:t0 + tn],
                                             start=(r == 0 and kp == 0),
                                             stop=(r == 2 and kp == 3),
                                             perf_mode=DR)
                            if kp % 2 == 1 and (r, kp) != (2, 3):
                                yield
                    if which == "q":
                        nc.vector.tensor_scalar_add(dst[:, m, tsl], p[:, 0:tn],
                                                    bq_sb[:, m:m + 1])
                    else:
                        nc.vector.tensor_copy(dst[:, m, tsl], p[:, 0:tn])
                return go()

            def unit_v(t4, si):
                def go():
                    x8, xr, xd = xsb_tiles[t4]
                    tl = slice(128 * si, 128 * si + 128)
                    p = ps.tile([128, DL], F32, tag="mm", bufs=2,
                                name=f"v{t4}{si}")
                    for r, xop in ((0, x8), (1, xr), (2, xd)):
                        for kp in range(4):
                            kk = slice(2 * kp, 2 * kp + 2)
                            nc.tensor.matmul(p[:, :], xop[:, kk, tl],
                                             wv_sb[:, kk, r, :],
                                             start=(r == 0 and kp == 0),
                                             stop=(r == 2 and kp == 3),
                                             perf_mode=DR)
                        if r < 2:
                            yield
                    v3 = vO_sb[4 * t4 + si]
                    nc.vector.tensor_copy(
                        v3[:, :, 0:HD], p[:].rearrange("p (h c) -> p h c", h=HL))
                return go()

            def unit_pj(j, sp, on_act=False):
                """Project 256 tokens; fp16 partials out. on_act: route the
                psum->sbuf copies to ACT (for the tail, when exps are done)."""
                def go():
                    t0 = 512 * j + 256 * sp
                    osb = outp.tile([128, 2, D], F16, tag="o", name=f"o{j}{sp}")
                    for sub in range(2):
                        tsl = slice(t0 + 128 * sub, t0 + 128 * sub + 128)
                        for ncol in range(2):
                            pj = ps.tile([128, 512], F32, tag="mm", bufs=2,
                                         name=f"pj{j}{sp}{sub}{ncol}")
                            for m2 in range(2):
                                nc.tensor.matmul(
                                    pj[:, :], yT_sb[m2][:, tsl],
                                    wp_sb[:, m2, 512 * ncol:512 * ncol + 512],
                                    start=(m2 == 0), stop=(m2 == 1))
                            dst = osb[:, sub, 512 * ncol:512 * ncol + 512]
                            if on_act:
                                nc.scalar.activation(dst, pj, AF.Copy)
                            else:
                                nc.vector.tensor_copy(dst, pj)
                            yield
                        nc.scalar.dma_start(out=out[tsl, :],
                                            in_=osb[:, sub, :])
                return go()

            def unit_tr(j, on_act=False):
                """Transpose yQ chunks of block j into yT via PE is_transpose
                (borrows an mm psum slot bitcast to bf16; the XBAR DMA route
                clogs the HWDGE queue). on_act: tail variant — psum->sbuf
                copies go to ACT, which is idle once the exps are done."""
                def go():
                    for qc in range(4 * j, 4 * j + 4):
                        tp = ps.tile([128, 512], F32, tag="mm", bufs=2,
                                     name=f"tp{qc}")
                        tpb = tp.bitcast(BF16)
                        for m in range(2):
                            nc.tensor.transpose(
                                tpb[:, 128 * m:128 * m + 128],
                                yQ_sb[:, qc, 128 * m:128 * m + 128], id_sb)
                        for m in range(2):
                            dst = yT_sb[m][:, 128 * qc:128 * qc + 128]
                            src = tpb[:, 128 * m:128 * m + 128]
                            if on_act:
                                nc.scalar.activation(dst, src, AF.Copy)
                            else:
                                nc.vector.tensor_copy(dst, src)
                        yield
                return go()

            def drain(gens):
                for g in gens:
                    for _ in g:
                        pass

            # prologue: block 0's q, then only the first 256 key-tokens of k
            # — enough for the first score pair; the second k half follows
            # while that pair's exp runs
            drain([unit_qk("q", 0, 0), unit_qk("q", 0, 1),
                   unit_qk("k", 0, 0, 0, 256), unit_qk("k", 0, 1, 0, 256)])
            k2nd = [unit_qk("k", 0, 0, 256, 256), unit_qk("k", 0, 1, 256, 256)]

            for j in range(4):
                if 2 <= j + 1 < 4:
                    dma_x(j + 1)
                npairs = 2 * (j + 1)
                # rolling filler queue (FIFO of generators); gens with
                # deadlines are tracked by name and force-completed in time
                vgens = [unit_v(j, si) for si in range(4)]
                queue = []
                if j > 0:
                    queue.append(unit_tr(j - 1))
                queue += vgens
                if j > 0:
                    queue += [unit_pj(j - 1, 0), unit_pj(j - 1, 1)]
                if j + 1 < 4:
                    queue += [unit_qk(w, j + 1, m)
                              for w in ("q", "k") for m in range(2)]
                nsteps = {0: 36, 1: 16, 2: 24, 3: 27}[j]
                rate = -(-nsteps // (HL * npairs))
                done = set()

                def pump(steps):
                    while steps > 0 and queue:
                        g = queue[0]
                        try:
                            next(g)
                            steps -= 1
                        except StopIteration:
                            done.add(g)
                            queue.pop(0)

                def force(gens):
                    for g in gens:
                        if g in done:
                            continue
                        for _ in g:
                            pass
                        done.add(g)
                        if g in queue:
                            queue.remove(g)

                def emit_st(h, p):
                    # pair p covers key chunks (2p, 2p+1)
                    hp = slice(32 * h, 32 * h + 32)
                    diag = p >= 2 * j
                    qs = 512 * j if p <= 2 * j else 512 * j + 256
                    n = 512 * j + 512 - qs
                    st = ps.tile([128, 2, 512], F32, tag="st", bufs=2,
                                 name=f"st{j}{h}{p}")
                    for c in range(2):
                        ki = 128 * (2 * p + c)
                        nc.tensor.matmul(st[:, c, 0:n],
                                         kT_sb[hp, :, ki:ki + 128],
                                         qT_sb[hp, :, qs:qs + n],
                                         start=True, stop=True,
                                         perf_mode=DR,
                                         tile_position=(32 * h, 0))
                    pt = ptp.tile([128, 2, 512], BF16, tag="pt",
                                  name=f"pt{j}{h}{p}")
                    nc.scalar.activation(pt[:, :, 0:n], st[:, :, 0:n],
                                         AF.Exp, scale=EXP_SCALE)
                    if diag:
                        # 0/1 masks post-exp, on Pool (keeps ACT/DVE free)
                        nc.gpsimd.tensor_tensor(
                            pt[:, 0, 0:128], pt[:, 0, 0:128], tri_sb,
                            op=OP.mult)
                        nc.gpsimd.tensor_tensor(
                            pt[:, 1, 0:256], pt[:, 1, 0:256], triw_sb,
                            op=OP.mult)
                    return pt, qs, n

                def emit_ot(h, p, yps, pt, qs, n):
                    # flipped att@V: pt stationary, v+ones moving. The four
                    # query-sub accumulation groups share one psum bank;
                    # start=True zeroes the WHOLE 2KB bank (ZERO_REGION_SIZE),
                    # so only the bank's first matmul sets it — the other
                    # groups' first writes land on pending-zero bytes and
                    # overwrite, then accumulate.
                    for qc in range(4):
                        for c in range(2):
                            i = 2 * p + c
                            if i > 4 * j + qc:
                                continue
                            off = 128 * qc + 512 * j - qs
                            if off < 0:
                                continue
                            nc.tensor.matmul(
                                yps[:, qc, 0:HD + 1],
                                pt[:, c, off:off + 128],
                                vO_sb[i][:, h, :],
                                start=(i == 0 and qc == 0),
                                stop=(i == 4 * j + qc),
                                skip_group_check=True)

                def emit_norm(h, yps):
                    # col 64 of each group is the denominator; token-major
                    # layout makes this a per-partition scalar multiply
                    rb = rcp.tile([128, 4], F32, tag=f"rb{h}", name=f"rb{j}{h}")
                    nc.vector.reciprocal(
                        rb, yps[:, :, HD:HD + 1].rearrange("p a b -> p (a b)"))
                    for qc in range(4):
                        nc.vector.tensor_scalar_mul(
                            yQ_sb[:, 4 * j + qc, 64 * h:64 * h + 64],
                            yps[:, qc, 0:HD], rb[:, qc:qc + 1])

                # one flat software-pipelined stream of (h, pair) across all
                # heads: no ACT bubble at h boundaries
                slots = [(h, p) for h in range(HL) for p in range(npairs)]
                LOOK = 2
                pts = {}
                yps_h = {}
                pts[0] = emit_st(*slots[0])
                if j == 0:  # second k half lands while pair 0's exp runs
                    drain(k2nd)
                pts[1] = emit_st(*slots[1])
                for idx, (h, p) in enumerate(slots):
                    if idx + LOOK < len(slots):
                        pts[idx + LOOK] = emit_st(*slots[idx + LOOK])
                    if h == 0:
                        if p == 2 * j:
                            force(vgens[:2])
                        elif p == 2 * j + 1:
                            force(vgens[2:])
                    if p == 0:
                        yps_h[h] = ps.tile([128, 4, 128], F32, tag="yps",
                                           bufs=2, name=f"yps{j}{h}")
                    emit_ot(h, p, yps_h[h], *pts.pop(idx))
                    if p == npairs - 1:
                        emit_norm(h, yps_h.pop(h))
                    if j > 0 or h > 0:  # j0-h0: x1 hasn't landed yet
                        pump(rate)
                pump(10 ** 9)  # flush fillers before the next j block
            drain([unit_tr(3), unit_pj(3, 0), unit_pj(3, 1)])

    _install_legalizer(nc)
    return nc


_NC_CACHE = None


def _get_nc():
    global _NC_CACHE
    if _NC_CACHE is None:
        _NC_CACHE = build_nc()
    return _NC_CACHE


# low/high 32-dim halves of each head -> planes (col 128c+32h+p of the
# permuted weight = dim 64h+32c+p of head h)
_PERM = np.array([64 * h + 32 * c + p
                  for c in range(2) for h in range(4) for p in range(32)])


def _comp8(a):
    """(fp8(a), fp8(a8/16), fp8(16*(a-a8))): one-psum compensated operands."""
    a = np.asarray(a, np.float32)
    a8 = a.astype(NP_FP8)
    a8f = a8.astype(np.float32)
    return a8, (a8f / 16.0).astype(NP_FP8), ((a - a8f) * 16.0).astype(NP_FP8)


def make_in_maps(x, Wq, bq, Wk, Wv, Wp):
    x = np.asarray(x, np.float32)
    xs = [_comp8(np.ascontiguousarray(x[b].T)) for b in range(2)]
    t01 = (np.arange(128)[None, :] >= np.arange(128)[:, None])
    tri01 = t01.astype(NP_BF16)
    triw01 = np.concatenate(
        [np.zeros((128, 128), NP_BF16), tri01], axis=1)
    Wq, Wk, Wv, Wp = (np.asarray(w, np.float32) for w in (Wq, Wk, Wv, Wp))
    bq = np.asarray(bq, np.float32)

    def wpack(wg):  # [DL, D] scaled -> [D, 3, DL] fp8 (w8 | w8/16 | 16*res)
        w8, wd, wr = _comp8(np.ascontiguousarray(wg.T))
        return np.ascontiguousarray(np.stack([w8, wd, wr], axis=1))

    in_maps = []
    for c in range(8):
        b, g = c // 4, c % 4
        sl = slice(DL * g, DL * g + DL)
        in_maps.append({
            "xT": xs[b][0],
            "xTd": xs[b][1],
            "xTr": xs[b][2],
            "wq": wpack(Wq[sl, :][_PERM, :] * WS),
            "wk": wpack(Wk[sl, :][_PERM, :] * WS),
            "wv": wpack(Wv[sl, :] * WS),
            "wp": np.ascontiguousarray(Wp[:, sl].T / WS).astype(NP_BF16),
            "bq": np.ascontiguousarray(bq[sl][_PERM]) * np.float32(WS),
            "tri": tri01,
            "triw": triw01,
            "ident": np.eye(128, dtype=np.float32).astype(NP_BF16),
        })
    return in_maps


def kernel(x, Wq, bq, Wk, bk, Wv, bv, Wp, bp, _run_kwargs=None):
    nc = _get_nc()
    in_maps = make_in_maps(x, Wq, bq, Wk, Wv, Wp)
    res = run_bass_kernel_spmd(nc, in_maps, list(range(8)), **(_run_kwargs or {}))
    corr = (np.asarray(bv, np.float32) @ np.asarray(Wp, np.float32).T
            + np.asarray(bp, np.float32))
    out = np.zeros((2, S, D), np.float32)
    for c in range(8):
        out[c // 4] += np.asarray(res.results[c]["out"], np.float32)
    out += corr[None, None, :]
    kernel.last_results = res
    return out
